# revision 36
# baseline (speedup 1.0000x reference)
"""CondLSTMProposal Trainium2 kernel.

Data-parallel over batch: 8 cores x 256 batch rows each. Everything on-chip
runs in transposed [feature, batch] layout so the LSTM recurrence needs no
transposes. Phases:
  P0  FiLM conditioning (fp32): gamT/betT/e0T per core.
  P1  teacher-forced inputs: one-hot(X) @ emb (pair-packed, bf16 matmul),
      FiLM-modulate, store xsT[t] to DRAM (bf16).
  P2  2-layer LSTM, 256 unrolled steps, bf16 matmuls into fp32 PSUM,
      sigmoid/tanh on ScalarE (gate order reordered to i,f,o,g so one
      sigmoid op covers a contiguous range), cell state fp32.
  P3  head: logits in [batch, t, k] layout via lhsT=h1T, exp/reduce/ln for
      LSE, iota-compare mask gather, logq = gather - lse.
"""

import sys

sys.path.insert(0, "/opt/trn_rl_repo")

import numpy as np
import ml_dtypes

import concourse.bass as bass
import concourse.bacc as bacc
import concourse.tile as tile
from concourse import mybir
from concourse.bass_utils import run_bass_kernel_spmd


def _install_ntff_hook_shim():
    """Provide antenv.axon_hooks (absent from this image) so trace=True works."""
    import types
    if "antenv.axon_hooks" in sys.modules:
        return
    mod = types.ModuleType("antenv.axon_hooks")
    state = {"hook": None}

    def set_axon_ntff_profile_hook(hook):
        state["hook"] = hook

    def get_axon_ntff_profile_hook():
        if state["hook"] is None:
            try:
                from trn_agent_boot.trn_boot import _ntff_profile_via_ctypes
                state["hook"] = _ntff_profile_via_ctypes("/opt/axon/libaxon_pjrt.so")
            except Exception:
                state["hook"] = None
        return state["hook"]

    mod.set_axon_ntff_profile_hook = set_axon_ntff_profile_hook
    mod.get_axon_ntff_profile_hook = get_axon_ntff_profile_hook
    sys.modules["antenv.axon_hooks"] = mod
    try:
        import antenv
        antenv.axon_hooks = mod
    except ImportError:
        pass


_install_ntff_hook_shim()

FP32 = mybir.dt.float32
BF16 = mybir.dt.bfloat16
FP8 = mybir.dt.float8e4
AF = mybir.ActivationFunctionType
ALU = mybir.AluOpType
BF = ml_dtypes.bfloat16
F8 = ml_dtypes.float8_e4m3fn
GS = 256.0                 # gate pre-activation scale (wh/wi fp8 x16, h0 fp8 x16)
HS = 16.0                  # h0 fp8 scale

B, D, K, UD, E, H = 2048, 256, 20, 512, 64, 256
HID = 512
N_CORES = 8
BL = B // N_CORES          # 256 per-core batch
G4 = 4 * H                 # 1024 gate rows
LN_EPS = 1e-5
TCH = 8                    # timesteps per P1/P3 chunk
NCH = D // TCH             # 32 chunks

CFG = {
    "debug": False,        # expose xs/hs/gam/bet as outputs
    "c_fp32": True,        # keep LSTM cell state in fp32
    "phases": "0123",     # which phases to emit (bisection aid)
    "nsteps": D,           # LSTM steps to emit
    "p1_level": 4,         # P1 sub-bisect: 1=dma+mask 2=+mm 3=+mod 4=+out
    "onesrow": False,      # (non-fp8 path) L0 bias via ones-row is WRONG with
                           # per-slot start=True: start marks the whole 2KB PSUM
                           # bank pending-zero, clobbering the sibling half-bank.
                           # fp8 path fixes this with N=1 zero-weight bank starts.
    "fp8": False,          # fp8 DoubleRow was a loss: DR disables FWL, matmuls
                           # got slower (301ns vs 2x107ns) and HAM went cold
    "dstart": True,        # g0: N=1 zero-weight bank starts + ones-row bias
                           # (replaces 4 N=512 bias matmuls per iteration)
}

_CACHE = {}


def _dma(nc, out, in_):
    nc.sync.dma_start(out=out, in_=in_)


def _bcast_dma(nc, out, in_ap):
    # partition-broadcast / fancy-AP DMAs go through gpsimd (SWDGE)
    nc.gpsimd.dma_start(out=out, in_=in_ap)


def _ap(handle, offset, dims):
    base = handle[tuple(slice(None) for _ in handle.shape)]
    return bass.AP(tensor=base.tensor, offset=offset, ap=[list(d) for d in dims])


def build_nc(cfg):
    nc = bacc.Bacc("TRN2")

    # ---- per-core inputs
    ut_d = nc.dram_tensor("ut", [UD, BL], FP32, kind="ExternalInput")
    xt_d = nc.dram_tensor("xt", [D, BL], BF16, kind="ExternalInput")
    xb_d = nc.dram_tensor("xb", [128, 2, D], BF16, kind="ExternalInput")

    # ---- replicated weights / constants
    gw1t_d = nc.dram_tensor("gw1t", [UD, HID], FP32, kind="ExternalInput")
    bw1t_d = nc.dram_tensor("bw1t", [UD, HID], FP32, kind="ExternalInput")
    gb1_d = nc.dram_tensor("gb1", [HID], FP32, kind="ExternalInput")
    bb1_d = nc.dram_tensor("bb1", [HID], FP32, kind="ExternalInput")
    glnw_d = nc.dram_tensor("glnw", [HID], FP32, kind="ExternalInput")
    glnb_d = nc.dram_tensor("glnb", [HID], FP32, kind="ExternalInput")
    blnw_d = nc.dram_tensor("blnw", [HID], FP32, kind="ExternalInput")
    blnb_d = nc.dram_tensor("blnb", [HID], FP32, kind="ExternalInput")
    gw2t_d = nc.dram_tensor("gw2t", [HID, E], FP32, kind="ExternalInput")
    bw2t_d = nc.dram_tensor("bw2t", [HID, E], FP32, kind="ExternalInput")
    gb2d_d = nc.dram_tensor("gb2d", [128, 1], FP32, kind="ExternalInput")
    bb2d_d = nc.dram_tensor("bb2d", [128, 1], FP32, kind="ExternalInput")
    u0wt_d = nc.dram_tensor("u0wt", [UD, E], FP32, kind="ExternalInput")
    u0b_d = nc.dram_tensor("u0b", [E, 1], FP32, kind="ExternalInput")
    ident_d = nc.dram_tensor("ident", [128, 128], FP32, kind="ExternalInput")

    emb2_d = nc.dram_tensor("emb2", [40, 128], BF16, kind="ExternalInput")
    iota2_d = nc.dram_tensor("iota2", [40, 1], FP32, kind="ExternalInput")

    w0x_d = nc.dram_tensor("w0x", [E + 1, G4], BF16, kind="ExternalInput")
    wh0_d = nc.dram_tensor("wh0", [H, G4], BF16, kind="ExternalInput")
    wi1_d = nc.dram_tensor("wi1", [H, G4], BF16, kind="ExternalInput")
    wh1_d = nc.dram_tensor("wh1", [H, G4], BF16, kind="ExternalInput")
    w0xs_d = nc.dram_tensor("w0xs", [E + 1, G4], BF16, kind="ExternalInput")
    wh08_d = nc.dram_tensor("wh08", [128, 2, G4], FP8, kind="ExternalInput")
    wi18_d = nc.dram_tensor("wi18", [128, 2, G4], FP8, kind="ExternalInput")
    wh1s_d = nc.dram_tensor("wh1s", [128, 2, G4], BF16, kind="ExternalInput")
    b1ps_d = nc.dram_tensor("b1ps", [2, 4, 128], BF16, kind="ExternalInput")
    b0p_d = nc.dram_tensor("b0p", [2, 4, 128], BF16, kind="ExternalInput")
    b1p_d = nc.dram_tensor("b1p", [2, 4, 128], BF16, kind="ExternalInput")
    sel2_d = nc.dram_tensor("sel2", [2, 512], BF16, kind="ExternalInput")

    hwt_d = nc.dram_tensor("hwt", [H, 20], BF16, kind="ExternalInput")
    hb8_d = nc.dram_tensor("hb8", [1, TCH * 20], BF16, kind="ExternalInput")
    hb16_d = nc.dram_tensor("hb16", [1, 2 * TCH * 20], BF16, kind="ExternalInput")
    iotak_d = nc.dram_tensor("iotak", [128, 20], BF16, kind="ExternalInput")

    # ---- outputs / scratch
    logq_d = nc.dram_tensor("logq", [BL], FP32, kind="ExternalOutput")
    sk = "ExternalOutput" if cfg["debug"] else "Internal"
    xs_d = nc.dram_tensor("xs", [D + 1, E + 1, BL], BF16, kind=sk)
    hs_d = nc.dram_tensor("hs", [D, 2, 128, BL], BF16, kind=sk)
    # per-chunk hs tensors: P3's chunk loads depend only on that chunk's 8
    # stores (DRAM deps are tracked per-tensor), so they prefetch during P2
    hs_ds = [nc.dram_tensor(f"hsc{ci}", [TCH, 2, 128, BL], BF16, kind="Internal")
             for ci in range(NCH)]
    if cfg["debug"]:
        dbg_gam_d = nc.dram_tensor("dbg_gam", [128, 4, BL], FP32, kind="ExternalOutput")
        dbg_bet_d = nc.dram_tensor("dbg_bet", [128, 4, BL], FP32, kind="ExternalOutput")
        dbg_g0_d = nc.dram_tensor("dbg_g0", [128, 8, BL], FP32, kind="ExternalOutput")
        dbg_h0_d = nc.dram_tensor("dbg_h0", [D, 2, 128, BL], FP32, kind="ExternalOutput")

    cdt = FP32 if cfg["c_fp32"] else BF16

    with tile.TileContext(nc) as tc:
        with tc.tile_pool(name="glob", bufs=1) as glob:
            # persistent across phases
            gp4 = glob.tile([128, 4, BL], FP32)   # (1+gam) doubled over partitions, x4 pair slots
            bt4 = glob.tile([128, 4, BL], FP32)

            # =========================== P0: FiLM ===========================
            if "0" in cfg["phases"]:
              with tc.tile_pool(name="p0in", bufs=1) as pin, \
                 tc.tile_pool(name="p0t", bufs=2) as ptmp, \
                 tc.tile_pool(name="p0ps", bufs=2, space="PSUM") as pps:
                ut_sb = pin.tile([128, 4, BL], FP32)
                _dma(nc, ut_sb, ut_d[:, :].rearrange("(c p) b -> p c b", p=128))
                ident_sb = pin.tile([128, 128], FP32)
                _dma(nc, ident_sb, ident_d[:, :])
                eps_sb = pin.tile([128, 1], FP32)
                nc.vector.memset(eps_sb, LN_EPS)

                branches = [
                    (gw1t_d, gb1_d, glnw_d, glnb_d, gw2t_d, gb2d_d, gp4),
                    (bw1t_d, bb1_d, blnw_d, blnb_d, bw2t_d, bb2d_d, bt4),
                ]
                for br, (w1d, b1d, lnwd, lnbd, w2d, b2dd, dst) in enumerate(branches):
                    w1_sb = pin.tile([128, 4, HID], FP32, name=f"w1_{br}")
                    _dma(nc, w1_sb, w1d[:, :].rearrange("(c p) n -> p c n", p=128))
                    b1b_sb = pin.tile([128, HID], FP32, name=f"b1b_{br}")
                    _bcast_dma(nc, b1b_sb, _ap(b1d, 0, [[0, 128], [1, HID]]))
                    lnw_sb = pin.tile([128, HID], FP32, name=f"lnw_{br}")
                    _bcast_dma(nc, lnw_sb, _ap(lnwd, 0, [[0, 128], [1, HID]]))
                    lnb_sb = pin.tile([128, HID], FP32, name=f"lnb_{br}")
                    _bcast_dma(nc, lnb_sb, _ap(lnbd, 0, [[0, 128], [1, HID]]))
                    w2_sb = pin.tile([128, 4, E], FP32, name=f"w2_{br}")
                    _dma(nc, w2_sb, w2d[:, :].rearrange("(c p) e -> p c e", p=128))
                    b2_sb = pin.tile([128, 1], FP32, name=f"b2_{br}")
                    _dma(nc, b2_sb, b2dd[:, :])

                    sT = pin.tile([128, 4, BL], FP32, name=f"sT_{br}")
                    for mb in range(2):
                        ps_h = pps.tile([128, HID], FP32, tag="ps_h")
                        for c in range(4):
                            nc.tensor.matmul(
                                ps_h, lhsT=ut_sb[:, c, mb * 128:(mb + 1) * 128],
                                rhs=w1_sb[:, c, :], start=(c == 0), stop=(c == 3))
                        h_sb = ptmp.tile([128, HID], FP32, tag="h_sb")
                        nc.vector.tensor_add(h_sb, ps_h, b1b_sb)
                        stats = ptmp.tile([128, 6], FP32, tag="stats")
                        nc.vector.bn_stats(out=stats, in_=h_sb)
                        mv = ptmp.tile([128, 2], FP32, tag="mv")
                        nc.vector.bn_aggr(out=mv, in_=stats)
                        std = ptmp.tile([128, 1], FP32, tag="std")
                        nc.scalar.activation(std, mv[:, 1:2], AF.Sqrt, bias=eps_sb)
                        rstd = ptmp.tile([128, 1], FP32, tag="rstd")
                        nc.vector.reciprocal(rstd, std)
                        nc.vector.tensor_scalar(
                            out=h_sb, in0=h_sb, scalar1=mv[:, 0:1], scalar2=rstd,
                            op0=ALU.subtract, op1=ALU.mult)
                        nc.vector.tensor_mul(h_sb, h_sb, lnw_sb)
                        nc.vector.tensor_add(h_sb, h_sb, lnb_sb)
                        s_sb = ptmp.tile([128, HID], FP32, tag="s_sb")
                        nc.scalar.activation(s_sb, h_sb, AF.Sigmoid)
                        nc.vector.tensor_mul(s_sb, s_sb, h_sb)
                        for c in range(4):
                            ps_t = pps.tile([128, 128], FP32, tag="ps_t")
                            nc.tensor.transpose(ps_t, s_sb[:, c * 128:(c + 1) * 128], ident_sb)
                            nc.scalar.copy(sT[:, c, mb * 128:(mb + 1) * 128], ps_t)

                    ps_o = pps.tile([128, BL], FP32, tag="ps_o")
                    for hf in range(2):
                        for c in range(4):
                            nc.tensor.matmul(
                                ps_o[hf * 64:(hf + 1) * 64, :],
                                lhsT=w2_sb[:, c, :], rhs=sT[:, c, :],
                                start=(c == 0), stop=(c == 3))
                    for j in range(4):
                        if br == 0:
                            nc.vector.tensor_scalar(
                                out=dst[:, j, :], in0=ps_o, scalar1=b2_sb, scalar2=1.0,
                                op0=ALU.add, op1=ALU.add)
                        else:
                            nc.vector.tensor_scalar_add(out=dst[:, j, :], in0=ps_o, scalar1=b2_sb)

                    if cfg["debug"]:
                        dd = dbg_gam_d if br == 0 else dbg_bet_d
                        _dma(nc, dd[:, :, :], dst[:, :, :])

                # e0T -> xs[0]
                u0w_sb = pin.tile([128, 4, E], FP32)
                _dma(nc, u0w_sb, u0wt_d[:, :].rearrange("(c p) e -> p c e", p=128))
                u0b_sb = pin.tile([E, 1], FP32)
                _dma(nc, u0b_sb, u0b_d[:, :])
                ps_e0 = pps.tile([E, BL], FP32, tag="ps_e0")
                for c in range(4):
                    nc.tensor.matmul(ps_e0, lhsT=u0w_sb[:, c, :], rhs=ut_sb[:, c, :],
                                     start=(c == 0), stop=(c == 3))
                e0bf = ptmp.tile([E, BL], BF16, tag="e0bf")
                nc.vector.tensor_scalar_add(out=e0bf, in0=ps_e0, scalar1=u0b_sb)
                _dma(nc, xs_d[0, 0:E, :], e0bf)

            # =========================== P1: xs =============================
            if "1" in cfg["phases"]:
              with tc.tile_pool(name="p1in", bufs=1) as pin, \
                 tc.tile_pool(name="p1t", bufs=3) as ptmp, \
                 tc.tile_pool(name="p1ps", bufs=2, space="PSUM") as pps:
                emb2_sb = pin.tile([40, 128], BF16)
                _dma(nc, emb2_sb, emb2_d[:, :])
                iota2_sb = pin.tile([40, 1], FP32)
                _dma(nc, iota2_sb, iota2_d[:, :])
                for ci in range(NCH):
                    t0 = ci * TCH
                    xb4 = ptmp.tile([40, 4, BL], BF16, tag="xb4")
                    for j in range(2):
                        _bcast_dma(nc, xb4[j * 20:(j + 1) * 20, :, :],
                                   _ap(xt_d, (t0 + j) * BL,
                                       [[0, 20], [2 * BL, 4], [1, BL]]))
                    mask = ptmp.tile([40, 4, BL], BF16, tag="m")
                    nc.vector.tensor_scalar(out=mask, in0=xb4, scalar1=iota2_sb,
                                            scalar2=None, op0=ALU.is_equal)
                    if cfg["p1_level"] < 2:
                        continue
                    ps_sel = pps.tile([128, 4, BL], FP32, tag="ps_sel")
                    for i in range(4):
                        nc.tensor.matmul(ps_sel[:, i, :], lhsT=emb2_sb, rhs=mask[:, i, :],
                                         start=True, stop=True)
                    if cfg["p1_level"] < 3:
                        xsb = ptmp.tile([128, 4, BL], BF16, tag="xsb")
                        nc.vector.tensor_copy(xsb, ps_sel)
                    else:
                        tmp = ptmp.tile([128, 4, BL], FP32, tag="tmp")
                        nc.vector.tensor_mul(tmp, ps_sel, gp4)
                        xsb = ptmp.tile([128, 4, BL], BF16, tag="xsb")
                        nc.vector.tensor_add(xsb, tmp, bt4)
                    if cfg["p1_level"] < 4:
                        continue
                    _dma(nc, _ap(xs_d, (t0 + 1) * (E + 1) * BL,
                                 [[BL, E], [2 * (E + 1) * BL, 4], [1, BL]]), xsb[0:E, :, :])
                    _dma(nc, _ap(xs_d, (t0 + 2) * (E + 1) * BL,
                                 [[BL, E], [2 * (E + 1) * BL, 4], [1, BL]]), xsb[E:128, :, :])

            # =========================== P2: LSTM ===========================
            # Software-pipelined: iteration it runs L0 step `it` and L1 step
            # `it-1`, so every matmul in an iteration reads only state written
            # in earlier iterations and the PE never stalls mid-iteration
            # (keeps the HAM clock-gate warm). L0 bias rides the ones-row of
            # xs (w0x row E); L1 bias stays as K=2 matmuls.
            if "2" in cfg["phases"]:
              with tc.tile_pool(name="p2w", bufs=1) as pw, \
                 tc.tile_pool(name="p2x", bufs=3) as px, \
                 tc.tile_pool(name="p2t", bufs=2) as pt, \
                 tc.tile_pool(name="p2g0", bufs=1, space="PSUM") as pg0, \
                 tc.tile_pool(name="p2g1", bufs=1, space="PSUM") as pg1:
                fp8 = cfg["fp8"]
                w0x_sb = pw.tile([E + 1, G4], BF16)
                _dma(nc, w0x_sb, (w0xs_d if fp8 else w0x_d)[:, :])
                ones_sb = pw.tile([128, BL], BF16)
                nc.vector.memset(ones_sb, 1.0)
                for tb in range(2):
                    _dma(nc, _ap(xs_d, (tb * 128 * (E + 1) + E) * BL,
                                 [[(E + 1) * BL, 128], [1, BL]]), ones_sb)
                if fp8:
                    wh0_sb = pw.tile([128, 2, G4], FP8)
                    _dma(nc, wh0_sb, wh08_d[:, :, :])
                    wi1_sb = pw.tile([128, 2, G4], FP8)
                    _dma(nc, wi1_sb, wi18_d[:, :, :])
                    wh1_sb = pw.tile([128, 2, G4], BF16)
                    _dma(nc, wh1_sb, wh1s_d[:, :, :])
                    b1p_sb = pw.tile([2, 4, 128], BF16)
                    _dma(nc, b1p_sb, b1ps_d[:, :, :])
                    zw_sb = pw.tile([1, 128], BF16)
                    nc.vector.memset(zw_sb, 0.0)
                else:
                    wh0_sb = pw.tile([128, 2, G4], BF16)
                    _dma(nc, wh0_sb, wh0_d[:, :].rearrange("(c p) n -> p c n", p=128))
                    wi1_sb = pw.tile([128, 2, G4], BF16)
                    _dma(nc, wi1_sb, wi1_d[:, :].rearrange("(c p) n -> p c n", p=128))
                    wh1_sb = pw.tile([128, 2, G4], BF16)
                    _dma(nc, wh1_sb, wh1_d[:, :].rearrange("(c p) n -> p c n", p=128))
                    b1p_sb = pw.tile([2, 4, 128], BF16)
                    _dma(nc, b1p_sb, b1p_d[:, :, :])
                    if cfg["dstart"]:
                        zw_sb = pw.tile([1, 128], BF16)
                        nc.vector.memset(zw_sb, 0.0)
                    else:
                        b0p_sb = pw.tile([2, 4, 128], BF16)
                        _dma(nc, b0p_sb, b0p_d[:, :, :])
                sel2_sb = pw.tile([2, 512], BF16)
                _dma(nc, sel2_sb, sel2_d[:, :])

                h0T = pw.tile([128, 2, BL], FP8 if fp8 else BF16)
                h1T = pw.tile([128, 2, BL], BF16)
                c0 = pw.tile([128, 2, BL], cdt)
                c1 = pw.tile([128, 2, BL], cdt)
                nc.vector.memset(h0T, 0.0)
                nc.vector.memset(h1T, 0.0)
                nc.vector.memset(c0, 0.0)
                nc.vector.memset(c1, 0.0)

                inv_gs = 1.0 / GS if fp8 else 1.0

                def lstm_layer(g, sigp, hT, cT, h_scale):
                    # gate nonlinearity + cell update; gates in g ([128,8,BL] psum)
                    # pre-activations are scaled by GS in the fp8 path
                    sig = sigp.tile([128, 6, BL], BF16, tag="sig")
                    nc.scalar.activation(sig, g[:, 0:6, :], AF.Sigmoid, scale=inv_gs)
                    tg = sigp.tile([128, 2, BL], BF16, tag="tg")
                    nc.scalar.activation(tg, g[:, 6:8, :], AF.Tanh, scale=inv_gs)
                    t1 = sigp.tile([128, 2, BL], BF16, tag="t1")
                    nc.vector.tensor_mul(t1, sig[:, 0:2, :], tg)
                    nc.vector.tensor_mul(cT, sig[:, 2:4, :], cT)
                    nc.vector.tensor_add(cT, cT, t1)
                    tcc = sigp.tile([128, 2, BL], BF16, tag="tcc")
                    nc.scalar.activation(tcc, cT, AF.Tanh)
                    if h_scale == 1.0:
                        nc.vector.tensor_mul(hT, sig[:, 4:6, :], tcc)
                    else:
                        nc.vector.scalar_tensor_tensor(
                            out=hT, in0=sig[:, 4:6, :], scalar=h_scale, in1=tcc,
                            op0=ALU.mult, op1=ALU.mult)

                nsteps = cfg["nsteps"]
                for it in range(nsteps + 1):
                    do0 = it < nsteps
                    do1 = it > 0

                    # --- tensor queue: everything reads prev-iter state only
                    if do0:
                        kx = E + 1 if (fp8 or cfg["dstart"]) else E
                        xin = px.tile([kx, BL], BF16, tag="xin")
                        _dma(nc, xin, xs_d[it, 0:kx, :])
                        g0 = pg0.tile([128, 8, BL], FP32, tag="g0")
                        if fp8:
                            # N=1 zero-weight matmuls legally start each 2KB bank
                            for bk in range(4):
                                nc.tensor.matmul(g0[:, 2 * bk, 0:1], lhsT=zw_sb,
                                                 rhs=ones_sb[0:1, 0:1], start=True,
                                                 stop=False, skip_group_check=True)
                            for m in range(8):
                                nc.tensor.matmul(g0[:, m, :],
                                                 lhsT=w0x_sb[:, m * 128:(m + 1) * 128],
                                                 rhs=xin, start=False, stop=False,
                                                 skip_group_check=True)
                            for m in range(8):
                                nc.tensor.matmul(
                                    g0[:, m, :],
                                    lhsT=wh0_sb[:, :, m * 128:(m + 1) * 128],
                                    rhs=h0T, start=False, stop=True,
                                    perf_mode=mybir.MatmulPerfMode.DoubleRow,
                                    skip_group_check=True)
                        elif cfg["dstart"]:
                            # one N=1 zero-weight matmul starts (pending-zeros)
                            # each 2KB PSUM bank; bias rides the xs ones-row
                            for bk in range(4):
                                nc.tensor.matmul(g0[:, 2 * bk, 0:1], lhsT=zw_sb,
                                                 rhs=ones_sb[0:1, 0:1], start=True,
                                                 stop=False, skip_group_check=True)
                            # slot-major so sigmoid's slots 0:6 finish early
                            for m in range(8):
                                nc.tensor.matmul(g0[:, m, :],
                                                 lhsT=w0x_sb[:, m * 128:(m + 1) * 128],
                                                 rhs=xin, start=False,
                                                 stop=False, skip_group_check=True)
                                for kc in range(2):
                                    nc.tensor.matmul(g0[:, m, :],
                                                     lhsT=wh0_sb[:, kc, m * 128:(m + 1) * 128],
                                                     rhs=h0T[:, kc, :], start=False,
                                                     stop=(kc == 1), skip_group_check=True)
                        else:
                            for bk in range(4):
                                nc.tensor.matmul(g0[:, 2 * bk:2 * bk + 2, :],
                                                 lhsT=b0p_sb[:, bk, :],
                                                 rhs=sel2_sb, start=True, stop=False,
                                                 skip_group_check=True)
                            for m in range(8):
                                nc.tensor.matmul(g0[:, m, :],
                                                 lhsT=w0x_sb[0:E, m * 128:(m + 1) * 128],
                                                 rhs=xin, start=False,
                                                 stop=False, skip_group_check=True)
                            for kc in range(2):
                                for m in range(8):
                                    nc.tensor.matmul(g0[:, m, :],
                                                     lhsT=wh0_sb[:, kc, m * 128:(m + 1) * 128],
                                                     rhs=h0T[:, kc, :], start=False,
                                                     stop=(kc == 1), skip_group_check=True)
                    if do1:
                        g1 = pg1.tile([128, 8, BL], FP32, tag="g1")
                        for bk in range(4):
                            nc.tensor.matmul(g1[:, 2 * bk:2 * bk + 2, :],
                                             lhsT=b1p_sb[:, bk, :],
                                             rhs=sel2_sb, start=True, stop=False,
                                             skip_group_check=True)
                        # wi1 @ h0(it-1) first (h0T still holds it-1 here)
                        if fp8:
                            for m in range(8):
                                nc.tensor.matmul(
                                    g1[:, m, :],
                                    lhsT=wi1_sb[:, :, m * 128:(m + 1) * 128],
                                    rhs=h0T, start=False, stop=False,
                                    perf_mode=mybir.MatmulPerfMode.DoubleRow,
                                    skip_group_check=True)
                        else:
                            for m in range(8):
                                for kc in range(2):
                                    nc.tensor.matmul(g1[:, m, :],
                                                     lhsT=wi1_sb[:, kc, m * 128:(m + 1) * 128],
                                                     rhs=h0T[:, kc, :], start=False,
                                                     stop=False, skip_group_check=True)
                        # wh1 @ h1(it-2): h1T write for it-1 lands late, so last
                        for m in range(8):
                            for kc in range(2):
                                nc.tensor.matmul(g1[:, m, :],
                                                 lhsT=wh1_sb[:, kc, m * 128:(m + 1) * 128],
                                                 rhs=h1T[:, kc, :], start=False,
                                                 stop=(kc == 1), skip_group_check=True)

                    # --- scalar/vector queues (overlap next iter's matmuls)
                    if do0:
                        if cfg["debug"] and it == 0:
                            g0cp = pt.tile([128, 8, BL], FP32, tag="g0cp")
                            nc.vector.tensor_copy(g0cp, g0)
                            _dma(nc, dbg_g0_d[:, :, :], g0cp)
                        lstm_layer(g0, pt, h0T, c0, HS if fp8 else 1.0)
                        if cfg["debug"]:
                            h0cp = pt.tile([128, 2, BL], FP32, tag="h0cp")
                            nc.vector.tensor_copy(h0cp, h0T)
                            _dma(nc, dbg_h0_d[it, :, :, :].rearrange("h p b -> p h b"),
                                 h0cp)
                    if do1:
                        lstm_layer(g1, pt, h1T, c1, 1.0)
                        if cfg["debug"]:
                            _dma(nc, hs_d[it - 1, :, :, :].rearrange("h p b -> p h b"),
                                 h1T)
                        else:
                            _dma(nc, hs_ds[(it - 1) // TCH][(it - 1) % TCH, :, :, :]
                                 .rearrange("h p b -> p h b"), h1T)

            # =========================== P3: head ===========================
            if "3" in cfg["phases"]:
              with tc.tile_pool(name="p3in", bufs=1) as pin, \
                 tc.tile_pool(name="p3t", bufs=3) as ptmp, \
                 tc.tile_pool(name="p3ps", bufs=3, space="PSUM") as pps:
                hwt_sb = pin.tile([128, 2, 20], BF16)
                _dma(nc, hwt_sb, hwt_d[:, :].rearrange("(c p) k -> p c k", p=128))
                hb16_sb = pin.tile([1, 2 * TCH * 20], BF16)
                _dma(nc, hb16_sb, hb16_d[:, :])
                ones1b = pin.tile([1, 128], BF16)
                nc.vector.memset(ones1b, 1.0)
                iotak_sb = pin.tile([128, 20], BF16)
                _dma(nc, iotak_sb, iotak_d[:, :])
                xb_sb = pin.tile([128, 2, D], BF16)
                _dma(nc, xb_sb, xb_d[:, :, :])
                # se values for all (hf, t): ln deferred to one batched pass so
                # the Exp/Ln activation tables are not reloaded per chunk
                se_all = pin.tile([128, 2, D], FP32)
                gth_all = pin.tile([128, 2, NCH], FP32)

                # one-hot(x) masks for ALL (hf, t, k) in a single broadcast
                # compare, off the per-chunk critical chain
                mask_all = pin.tile([128, 2, D, 20], BF16)
                xbc = bass.AP(tensor=xb_sb.tensor, offset=xb_sb.offset,
                              ap=[list(d) for d in xb_sb[:, :, :].ap] + [[0, 20]])
                iob = bass.AP(
                    tensor=iotak_sb.tensor, offset=iotak_sb.offset,
                    ap=[list(iotak_sb[:, :].ap[0]), [0, 2], [0, D], [1, 20]])
                nc.vector.tensor_tensor(mask_all, xbc, iob, ALU.is_equal)

                for ci in range(NCH):
                    t0 = ci * TCH
                    hsb = ptmp.tile([128, 2, TCH, BL], BF16, tag="hsb")
                    for kc in range(2):
                        # kc=0 on gpsimd so it prefetches during P2 (the sync
                        # queue is paced by P2's per-iteration stores)
                        eng = nc.gpsimd if kc == 0 else nc.sync
                        if cfg["debug"]:
                            src_ap = _ap(hs_d, (t0 * 2 + kc) * 128 * BL,
                                         [[BL, 128], [2 * 128 * BL, TCH], [1, BL]])
                        else:
                            src_ap = _ap(hs_ds[ci], kc * 128 * BL,
                                         [[BL, 128], [2 * 128 * BL, TCH], [1, BL]])
                        eng.dma_start(out=hsb[:, kc, :, :], in_=src_ap)
                    # one PSUM bank holds logits for both batch halves
                    ps_lg = pps.tile([128, 2, TCH, 20], FP32, tag="ps_lg")
                    nc.tensor.matmul(ps_lg[:, :, :, :], lhsT=ones1b, rhs=hb16_sb,
                                     start=True, stop=False, skip_group_check=True)
                    for hf in range(2):
                        for tt in range(TCH):
                            for kc in range(2):
                                nc.tensor.matmul(
                                    ps_lg[:, hf, tt, :],
                                    lhsT=hsb[:, kc, tt, hf * 128:(hf + 1) * 128],
                                    rhs=hwt_sb[:, kc, :],
                                    start=False, stop=(kc == 1), skip_group_check=True)
                    elg = ptmp.tile([128, 2, TCH, 20], FP32, tag="elg")
                    nc.scalar.activation(elg, ps_lg, AF.Exp)
                    nc.vector.tensor_reduce(se_all[:, :, t0:t0 + TCH], elg,
                                            axis=mybir.AxisListType.X, op=ALU.add)
                    # fused multiply+reduce of precomputed one-hot masks
                    # against the logits per half
                    scr = ptmp.tile([128, TCH, 20], FP32, tag="scr")
                    for hf in range(2):
                        nc.vector.scalar_tensor_tensor(
                            out=scr, in0=ps_lg[:, hf], scalar=1.0,
                            in1=mask_all[:, hf, t0:t0 + TCH, :],
                            op0=ALU.bypass, op1=ALU.mult,
                            accum_out=gth_all[:, hf, ci:ci + 1])

                lse2 = pin.tile([128, 2], FP32)
                for hf in range(2):
                    lnall = pin.tile([128, D], FP32, name=f"lnall_{hf}")
                    nc.scalar.activation(lnall, se_all[:, hf, :], AF.Ln,
                                         accum_out=lse2[:, hf:hf + 1])
                gth2 = pin.tile([128, 2], FP32)
                nc.vector.tensor_reduce(gth2, gth_all[:, :, :],
                                        axis=mybir.AxisListType.X, op=ALU.add)
                res = pin.tile([128, 2], FP32)
                nc.vector.tensor_sub(res, gth2, lse2)
                _dma(nc, _ap(logq_d, 0, [[1, 128], [128, 2]]), res)

    nc.compile()
    return nc


def _prep(inputs, cfg):
    f32 = np.float32
    emb = np.asarray(inputs["emb"], f32)
    perm = np.r_[0:2 * H, 3 * H:4 * H, 2 * H:3 * H]   # torch i,f,g,o -> i,f,o,g

    def pair_bias(b):
        # b[G4] -> [2, 4, 128]: b0p[j, bk, p] = b[(2bk+j)*128+p]
        return b.reshape(4, 2, 128).transpose(1, 0, 2).copy()

    wih0 = np.asarray(inputs["wih0"], f32)
    whh0 = np.asarray(inputs["whh0"], f32)
    wih1 = np.asarray(inputs["wih1"], f32)
    whh1 = np.asarray(inputs["whh1"], f32)
    b0 = (np.asarray(inputs["bih0"], f32) + np.asarray(inputs["bhh0"], f32))[perm]
    b1 = (np.asarray(inputs["bih1"], f32) + np.asarray(inputs["bhh1"], f32))[perm]

    emb2 = np.zeros((40, 128), f32)
    emb2[:20, :64] = emb
    emb2[20:, 64:] = emb
    sel2 = np.zeros((2, 512), f32)
    sel2[0, :256] = 1.0
    sel2[1, 256:] = 1.0
    gb2 = np.asarray(inputs["gb2"], f32)
    bb2 = np.asarray(inputs["bb2"], f32)

    shared = {
        "gw1t": np.ascontiguousarray(np.asarray(inputs["gw1"], f32).T),
        "bw1t": np.ascontiguousarray(np.asarray(inputs["bw1"], f32).T),
        "gb1": np.asarray(inputs["gb1"], f32),
        "bb1": np.asarray(inputs["bb1"], f32),
        "glnw": np.asarray(inputs["gln_w"], f32),
        "glnb": np.asarray(inputs["gln_b"], f32),
        "blnw": np.asarray(inputs["bln_w"], f32),
        "blnb": np.asarray(inputs["bln_b"], f32),
        "gw2t": np.ascontiguousarray(np.asarray(inputs["gw2"], f32).T),
        "bw2t": np.ascontiguousarray(np.asarray(inputs["bw2"], f32).T),
        "gb2d": np.concatenate([gb2, gb2])[:, None].copy(),
        "bb2d": np.concatenate([bb2, bb2])[:, None].copy(),
        "u0wt": np.ascontiguousarray(np.asarray(inputs["u0_w"], f32).T),
        "u0b": np.asarray(inputs["u0_b"], f32)[:, None].copy(),
        "ident": np.eye(128, dtype=f32),
        "emb2": emb2.astype(BF),
        "iota2": (np.arange(40, dtype=f32) % 20)[:, None].copy(),
        "w0x": np.ascontiguousarray(np.vstack([wih0.T[:, perm], b0[None, :]])).astype(BF),
        "wh0": np.ascontiguousarray(whh0.T[:, perm]).astype(BF),
        "wi1": np.ascontiguousarray(wih1.T[:, perm]).astype(BF),
        "wh1": np.ascontiguousarray(whh1.T[:, perm]).astype(BF),
        "b0p": pair_bias(b0).astype(BF),
        "b1p": pair_bias(b1).astype(BF),
        # fp8 path: gate pre-activations carry a GS=256 scale (wh0/wi1 are
        # fp8 x16 and h0 is stored as fp8 x16; bf16 contributions are x256)
        "w0xs": (np.vstack([wih0.T[:, perm], b0[None, :]]) * GS).astype(BF),
        "wh08": np.ascontiguousarray(
            (whh0.T[:, perm] * HS).reshape(2, 128, G4).transpose(1, 0, 2)).astype(F8),
        "wi18": np.ascontiguousarray(
            (wih1.T[:, perm] * HS).reshape(2, 128, G4).transpose(1, 0, 2)).astype(F8),
        "wh1s": np.ascontiguousarray(
            (whh1.T[:, perm] * GS).reshape(2, 128, G4).transpose(1, 0, 2)).astype(BF),
        "b1ps": (pair_bias(b1) * GS).astype(BF),
        "sel2": sel2.astype(BF),
        "hwt": np.ascontiguousarray(np.asarray(inputs["head_w"], f32).T).astype(BF),
        "hb8": np.tile(np.asarray(inputs["head_b"], f32), TCH)[None, :].astype(BF),
        "hb16": np.tile(np.asarray(inputs["head_b"], f32), 2 * TCH)[None, :].astype(BF),
        "iotak": np.tile(np.arange(20, dtype=f32), (128, 1)).astype(BF),
    }

    U = np.asarray(inputs["U"], f32)
    X = np.asarray(inputs["X"])
    in_maps = []
    for c in range(N_CORES):
        Us = U[c * BL:(c + 1) * BL]
        Xs = X[c * BL:(c + 1) * BL]
        m = dict(shared)
        m["ut"] = np.ascontiguousarray(Us.T)
        m["xt"] = np.ascontiguousarray(Xs.T.astype(f32)).astype(BF)
        m["xb"] = np.ascontiguousarray(
            Xs.astype(f32).reshape(2, 128, D).transpose(1, 0, 2)).astype(BF)
        in_maps.append(m)
    return in_maps


def get_nc(cfg=None):
    cfg = dict(CFG, **(cfg or {}))
    key = tuple(sorted(cfg.items()))
    if key not in _CACHE:
        _CACHE[key] = build_nc(cfg)
    return _CACHE[key]


def run(inputs, cfg=None, **run_kwargs):
    cfg = dict(CFG, **(cfg or {}))
    nc = get_nc(cfg)
    in_maps = _prep(inputs, cfg)
    res = run_bass_kernel_spmd(nc, in_maps, core_ids=list(range(N_CORES)), **run_kwargs)
    return res


def kernel(**inputs):
    res = run(inputs)
    out = np.concatenate([res.results[c]["logq"] for c in range(N_CORES)])
    return out.astype(np.float32)



# revision 38
# speedup vs baseline: 1.0044x; 1.0044x over previous
"""CondLSTMProposal Trainium2 kernel.

Data-parallel over batch: 8 cores x 256 batch rows each. Everything on-chip
runs in transposed [feature, batch] layout so the LSTM recurrence needs no
transposes. Phases:
  P0  FiLM conditioning (fp32): gamT/betT/e0T per core.
  P1  teacher-forced inputs: one-hot(X) @ emb (pair-packed, bf16 matmul),
      FiLM-modulate, store xsT[t] to DRAM (bf16).
  P2  2-layer LSTM, software-pipelined: iteration `it` runs layer0 step `it`
      and layer1 step `it-1`, so every matmul of an iteration reads only
      previous-iteration state and the PE issue stream never stalls (keeps
      the HAM clock-gate warm at 8/8 - this alone is worth ~1.4x). Layer-0
      PSUM banks are started by N=1 zero-weight matmuls with the bias riding
      the xs ones-row (a start=True matmul marks its whole 2KB PSUM bank
      pending-zero, so each bank must be started exactly once, bank-wide).
      bf16 matmuls into fp32 PSUM, bf16 activations, fp32 cell state.
  P3  head: logits in [batch, t, k] layout, one PSUM bank per 8-step chunk
      covering both batch halves; exp per chunk, ln deferred to one batched
      pass (avoids per-chunk Exp/Ln activation-table reloads); one-hot
      gather via a 0-stride broadcast is_equal + scalar_tensor_tensor with
      accumulate; hs split into per-chunk DRAM tensors so chunk loads
      prefetch during P2.
"""

import sys

sys.path.insert(0, "/opt/trn_rl_repo")

import numpy as np
import ml_dtypes

import concourse.bass as bass
import concourse.bacc as bacc
import concourse.tile as tile
from concourse import mybir
from concourse.bass_utils import run_bass_kernel_spmd


def _install_ntff_hook_shim():
    """Provide antenv.axon_hooks (absent from this image) so trace=True works."""
    import types
    if "antenv.axon_hooks" in sys.modules:
        return
    mod = types.ModuleType("antenv.axon_hooks")
    state = {"hook": None}

    def set_axon_ntff_profile_hook(hook):
        state["hook"] = hook

    def get_axon_ntff_profile_hook():
        if state["hook"] is None:
            try:
                from trn_agent_boot.trn_boot import _ntff_profile_via_ctypes
                state["hook"] = _ntff_profile_via_ctypes("/opt/axon/libaxon_pjrt.so")
            except Exception:
                state["hook"] = None
        return state["hook"]

    mod.set_axon_ntff_profile_hook = set_axon_ntff_profile_hook
    mod.get_axon_ntff_profile_hook = get_axon_ntff_profile_hook
    sys.modules["antenv.axon_hooks"] = mod
    try:
        import antenv
        antenv.axon_hooks = mod
    except ImportError:
        pass


_install_ntff_hook_shim()

FP32 = mybir.dt.float32
BF16 = mybir.dt.bfloat16
FP8 = mybir.dt.float8e4
AF = mybir.ActivationFunctionType
ALU = mybir.AluOpType
BF = ml_dtypes.bfloat16
F8 = ml_dtypes.float8_e4m3fn
GS = 256.0                 # gate pre-activation scale (wh/wi fp8 x16, h0 fp8 x16)
HS = 16.0                  # h0 fp8 scale

B, D, K, UD, E, H = 2048, 256, 20, 512, 64, 256
HID = 512
N_CORES = 8
BL = B // N_CORES          # 256 per-core batch
G4 = 4 * H                 # 1024 gate rows
LN_EPS = 1e-5
TCH = 8                    # timesteps per P1/P3 chunk
NCH = D // TCH             # 32 chunks

CFG = {
    "debug": False,        # expose xs/hs/gam/bet as outputs
    "c_fp32": True,        # keep LSTM cell state in fp32
    "phases": "0123",     # which phases to emit (bisection aid)
    "nsteps": D,           # LSTM steps to emit
    "p1_level": 4,         # P1 sub-bisect: 1=dma+mask 2=+mm 3=+mod 4=+out
    "onesrow": False,      # (non-fp8 path) L0 bias via ones-row is WRONG with
                           # per-slot start=True: start marks the whole 2KB PSUM
                           # bank pending-zero, clobbering the sibling half-bank.
                           # fp8 path fixes this with N=1 zero-weight bank starts.
    "fp8": False,          # fp8 DoubleRow was a loss: DR disables FWL, matmuls
                           # got slower (301ns vs 2x107ns) and HAM went cold
    "dstart": True,        # g0: N=1 zero-weight bank starts + ones-row bias
                           # (replaces 4 N=512 bias matmuls per iteration)
}

_CACHE = {}


def _dma(nc, out, in_):
    nc.sync.dma_start(out=out, in_=in_)


def _bcast_dma(nc, out, in_ap):
    # partition-broadcast / fancy-AP DMAs go through gpsimd (SWDGE)
    nc.gpsimd.dma_start(out=out, in_=in_ap)


def _ap(handle, offset, dims):
    base = handle[tuple(slice(None) for _ in handle.shape)]
    return bass.AP(tensor=base.tensor, offset=offset, ap=[list(d) for d in dims])


def build_nc(cfg):
    nc = bacc.Bacc("TRN2")

    # ---- per-core inputs
    ut_d = nc.dram_tensor("ut", [UD, BL], FP32, kind="ExternalInput")
    xt_d = nc.dram_tensor("xt", [D, BL], BF16, kind="ExternalInput")
    xb_d = nc.dram_tensor("xb", [128, 2, D], BF16, kind="ExternalInput")

    # ---- replicated weights / constants
    gw1t_d = nc.dram_tensor("gw1t", [UD, HID], FP32, kind="ExternalInput")
    bw1t_d = nc.dram_tensor("bw1t", [UD, HID], FP32, kind="ExternalInput")
    gb1_d = nc.dram_tensor("gb1", [HID], FP32, kind="ExternalInput")
    bb1_d = nc.dram_tensor("bb1", [HID], FP32, kind="ExternalInput")
    glnw_d = nc.dram_tensor("glnw", [HID], FP32, kind="ExternalInput")
    glnb_d = nc.dram_tensor("glnb", [HID], FP32, kind="ExternalInput")
    blnw_d = nc.dram_tensor("blnw", [HID], FP32, kind="ExternalInput")
    blnb_d = nc.dram_tensor("blnb", [HID], FP32, kind="ExternalInput")
    gw2t_d = nc.dram_tensor("gw2t", [HID, E], FP32, kind="ExternalInput")
    bw2t_d = nc.dram_tensor("bw2t", [HID, E], FP32, kind="ExternalInput")
    gb2d_d = nc.dram_tensor("gb2d", [128, 1], FP32, kind="ExternalInput")
    bb2d_d = nc.dram_tensor("bb2d", [128, 1], FP32, kind="ExternalInput")
    u0wt_d = nc.dram_tensor("u0wt", [UD, E], FP32, kind="ExternalInput")
    u0b_d = nc.dram_tensor("u0b", [E, 1], FP32, kind="ExternalInput")
    ident_d = nc.dram_tensor("ident", [128, 128], FP32, kind="ExternalInput")

    emb2_d = nc.dram_tensor("emb2", [40, 128], BF16, kind="ExternalInput")
    iota2_d = nc.dram_tensor("iota2", [40, 1], FP32, kind="ExternalInput")

    w0x_d = nc.dram_tensor("w0x", [E + 1, G4], BF16, kind="ExternalInput")
    wh0_d = nc.dram_tensor("wh0", [H, G4], BF16, kind="ExternalInput")
    wi1_d = nc.dram_tensor("wi1", [H, G4], BF16, kind="ExternalInput")
    wh1_d = nc.dram_tensor("wh1", [H, G4], BF16, kind="ExternalInput")
    w0xs_d = nc.dram_tensor("w0xs", [E + 1, G4], BF16, kind="ExternalInput")
    wh08_d = nc.dram_tensor("wh08", [128, 2, G4], FP8, kind="ExternalInput")
    wi18_d = nc.dram_tensor("wi18", [128, 2, G4], FP8, kind="ExternalInput")
    wh1s_d = nc.dram_tensor("wh1s", [128, 2, G4], BF16, kind="ExternalInput")
    b1ps_d = nc.dram_tensor("b1ps", [2, 4, 128], BF16, kind="ExternalInput")
    b0p_d = nc.dram_tensor("b0p", [2, 4, 128], BF16, kind="ExternalInput")
    b1p_d = nc.dram_tensor("b1p", [2, 4, 128], BF16, kind="ExternalInput")
    sel2_d = nc.dram_tensor("sel2", [2, 512], BF16, kind="ExternalInput")

    hwt_d = nc.dram_tensor("hwt", [H, 20], BF16, kind="ExternalInput")
    hb8_d = nc.dram_tensor("hb8", [1, TCH * 20], BF16, kind="ExternalInput")
    hb16_d = nc.dram_tensor("hb16", [1, 2 * TCH * 20], BF16, kind="ExternalInput")
    iotak_d = nc.dram_tensor("iotak", [128, 20], BF16, kind="ExternalInput")

    # ---- outputs / scratch
    logq_d = nc.dram_tensor("logq", [BL], FP32, kind="ExternalOutput")
    sk = "ExternalOutput" if cfg["debug"] else "Internal"
    xs_d = nc.dram_tensor("xs", [D + 1, E + 1, BL], BF16, kind=sk)
    hs_d = nc.dram_tensor("hs", [D, 2, 128, BL], BF16, kind=sk)
    # per-chunk hs tensors: P3's chunk loads depend only on that chunk's 8
    # stores (DRAM deps are tracked per-tensor), so they prefetch during P2
    hs_ds = [nc.dram_tensor(f"hsc{ci}", [TCH, 2, 128, BL], BF16, kind="Internal")
             for ci in range(NCH)]
    if cfg["debug"]:
        dbg_gam_d = nc.dram_tensor("dbg_gam", [128, 4, BL], FP32, kind="ExternalOutput")
        dbg_bet_d = nc.dram_tensor("dbg_bet", [128, 4, BL], FP32, kind="ExternalOutput")
        dbg_g0_d = nc.dram_tensor("dbg_g0", [128, 8, BL], FP32, kind="ExternalOutput")
        dbg_h0_d = nc.dram_tensor("dbg_h0", [D, 2, 128, BL], FP32, kind="ExternalOutput")

    cdt = FP32 if cfg["c_fp32"] else BF16

    with tile.TileContext(nc) as tc:
        with tc.tile_pool(name="glob", bufs=1) as glob:
            # persistent across phases
            gp4 = glob.tile([128, 4, BL], FP32)   # (1+gam) doubled over partitions, x4 pair slots
            bt4 = glob.tile([128, 4, BL], FP32)

            # =========================== P0: FiLM ===========================
            if "0" in cfg["phases"]:
              with tc.tile_pool(name="p0in", bufs=1) as pin, \
                 tc.tile_pool(name="p0t", bufs=2) as ptmp, \
                 tc.tile_pool(name="p0ps", bufs=2, space="PSUM") as pps:
                ut_sb = pin.tile([128, 4, BL], FP32)
                _dma(nc, ut_sb, ut_d[:, :].rearrange("(c p) b -> p c b", p=128))
                ident_sb = pin.tile([128, 128], FP32)
                _dma(nc, ident_sb, ident_d[:, :])
                eps_sb = pin.tile([128, 1], FP32)
                nc.vector.memset(eps_sb, LN_EPS)

                branches = [
                    (gw1t_d, gb1_d, glnw_d, glnb_d, gw2t_d, gb2d_d, gp4),
                    (bw1t_d, bb1_d, blnw_d, blnb_d, bw2t_d, bb2d_d, bt4),
                ]
                for br, (w1d, b1d, lnwd, lnbd, w2d, b2dd, dst) in enumerate(branches):
                    w1_sb = pin.tile([128, 4, HID], FP32, name=f"w1_{br}")
                    _dma(nc, w1_sb, w1d[:, :].rearrange("(c p) n -> p c n", p=128))
                    b1b_sb = pin.tile([128, HID], FP32, name=f"b1b_{br}")
                    _bcast_dma(nc, b1b_sb, _ap(b1d, 0, [[0, 128], [1, HID]]))
                    lnw_sb = pin.tile([128, HID], FP32, name=f"lnw_{br}")
                    _bcast_dma(nc, lnw_sb, _ap(lnwd, 0, [[0, 128], [1, HID]]))
                    lnb_sb = pin.tile([128, HID], FP32, name=f"lnb_{br}")
                    _bcast_dma(nc, lnb_sb, _ap(lnbd, 0, [[0, 128], [1, HID]]))
                    w2_sb = pin.tile([128, 4, E], FP32, name=f"w2_{br}")
                    _dma(nc, w2_sb, w2d[:, :].rearrange("(c p) e -> p c e", p=128))
                    b2_sb = pin.tile([128, 1], FP32, name=f"b2_{br}")
                    _dma(nc, b2_sb, b2dd[:, :])

                    sT = pin.tile([128, 4, BL], FP32, name=f"sT_{br}")
                    for mb in range(2):
                        ps_h = pps.tile([128, HID], FP32, tag="ps_h")
                        for c in range(4):
                            nc.tensor.matmul(
                                ps_h, lhsT=ut_sb[:, c, mb * 128:(mb + 1) * 128],
                                rhs=w1_sb[:, c, :], start=(c == 0), stop=(c == 3))
                        h_sb = ptmp.tile([128, HID], FP32, tag="h_sb")
                        nc.vector.tensor_add(h_sb, ps_h, b1b_sb)
                        stats = ptmp.tile([128, 6], FP32, tag="stats")
                        nc.vector.bn_stats(out=stats, in_=h_sb)
                        mv = ptmp.tile([128, 2], FP32, tag="mv")
                        nc.vector.bn_aggr(out=mv, in_=stats)
                        std = ptmp.tile([128, 1], FP32, tag="std")
                        nc.scalar.activation(std, mv[:, 1:2], AF.Sqrt, bias=eps_sb)
                        rstd = ptmp.tile([128, 1], FP32, tag="rstd")
                        nc.vector.reciprocal(rstd, std)
                        nc.vector.tensor_scalar(
                            out=h_sb, in0=h_sb, scalar1=mv[:, 0:1], scalar2=rstd,
                            op0=ALU.subtract, op1=ALU.mult)
                        nc.vector.tensor_mul(h_sb, h_sb, lnw_sb)
                        nc.vector.tensor_add(h_sb, h_sb, lnb_sb)
                        s_sb = ptmp.tile([128, HID], FP32, tag="s_sb")
                        nc.scalar.activation(s_sb, h_sb, AF.Sigmoid)
                        nc.vector.tensor_mul(s_sb, s_sb, h_sb)
                        for c in range(4):
                            ps_t = pps.tile([128, 128], FP32, tag="ps_t")
                            nc.tensor.transpose(ps_t, s_sb[:, c * 128:(c + 1) * 128], ident_sb)
                            nc.scalar.copy(sT[:, c, mb * 128:(mb + 1) * 128], ps_t)

                    ps_o = pps.tile([128, BL], FP32, tag="ps_o")
                    for hf in range(2):
                        for c in range(4):
                            nc.tensor.matmul(
                                ps_o[hf * 64:(hf + 1) * 64, :],
                                lhsT=w2_sb[:, c, :], rhs=sT[:, c, :],
                                start=(c == 0), stop=(c == 3))
                    for j in range(4):
                        if br == 0:
                            nc.vector.tensor_scalar(
                                out=dst[:, j, :], in0=ps_o, scalar1=b2_sb, scalar2=1.0,
                                op0=ALU.add, op1=ALU.add)
                        else:
                            nc.vector.tensor_scalar_add(out=dst[:, j, :], in0=ps_o, scalar1=b2_sb)

                    if cfg["debug"]:
                        dd = dbg_gam_d if br == 0 else dbg_bet_d
                        _dma(nc, dd[:, :, :], dst[:, :, :])

                # e0T -> xs[0]
                u0w_sb = pin.tile([128, 4, E], FP32)
                _dma(nc, u0w_sb, u0wt_d[:, :].rearrange("(c p) e -> p c e", p=128))
                u0b_sb = pin.tile([E, 1], FP32)
                _dma(nc, u0b_sb, u0b_d[:, :])
                ps_e0 = pps.tile([E, BL], FP32, tag="ps_e0")
                for c in range(4):
                    nc.tensor.matmul(ps_e0, lhsT=u0w_sb[:, c, :], rhs=ut_sb[:, c, :],
                                     start=(c == 0), stop=(c == 3))
                e0bf = ptmp.tile([E, BL], BF16, tag="e0bf")
                nc.vector.tensor_scalar_add(out=e0bf, in0=ps_e0, scalar1=u0b_sb)
                _dma(nc, xs_d[0, 0:E, :], e0bf)

            # =========================== P1: xs =============================
            if "1" in cfg["phases"]:
              with tc.tile_pool(name="p1in", bufs=1) as pin, \
                 tc.tile_pool(name="p1t", bufs=3) as ptmp, \
                 tc.tile_pool(name="p1ps", bufs=2, space="PSUM") as pps:
                emb2_sb = pin.tile([40, 128], BF16)
                _dma(nc, emb2_sb, emb2_d[:, :])
                iota2_sb = pin.tile([40, 1], FP32)
                _dma(nc, iota2_sb, iota2_d[:, :])
                for ci in range(NCH):
                    t0 = ci * TCH
                    xb4 = ptmp.tile([40, 4, BL], BF16, tag="xb4")
                    for j in range(2):
                        _bcast_dma(nc, xb4[j * 20:(j + 1) * 20, :, :],
                                   _ap(xt_d, (t0 + j) * BL,
                                       [[0, 20], [2 * BL, 4], [1, BL]]))
                    mask = ptmp.tile([40, 4, BL], BF16, tag="m")
                    nc.vector.tensor_scalar(out=mask, in0=xb4, scalar1=iota2_sb,
                                            scalar2=None, op0=ALU.is_equal)
                    if cfg["p1_level"] < 2:
                        continue
                    ps_sel = pps.tile([128, 4, BL], FP32, tag="ps_sel")
                    for i in range(4):
                        nc.tensor.matmul(ps_sel[:, i, :], lhsT=emb2_sb, rhs=mask[:, i, :],
                                         start=True, stop=True)
                    if cfg["p1_level"] < 3:
                        xsb = ptmp.tile([128, 4, BL], BF16, tag="xsb")
                        nc.vector.tensor_copy(xsb, ps_sel)
                    else:
                        tmp = ptmp.tile([128, 4, BL], FP32, tag="tmp")
                        nc.vector.tensor_mul(tmp, ps_sel, gp4)
                        xsb = ptmp.tile([128, 4, BL], BF16, tag="xsb")
                        nc.vector.tensor_add(xsb, tmp, bt4)
                    if cfg["p1_level"] < 4:
                        continue
                    _dma(nc, _ap(xs_d, (t0 + 1) * (E + 1) * BL,
                                 [[BL, E], [2 * (E + 1) * BL, 4], [1, BL]]), xsb[0:E, :, :])
                    _dma(nc, _ap(xs_d, (t0 + 2) * (E + 1) * BL,
                                 [[BL, E], [2 * (E + 1) * BL, 4], [1, BL]]), xsb[E:128, :, :])

            # =========================== P2: LSTM ===========================
            # Software-pipelined: iteration it runs L0 step `it` and L1 step
            # `it-1`, so every matmul in an iteration reads only state written
            # in earlier iterations and the PE never stalls mid-iteration
            # (keeps the HAM clock-gate warm). L0 bias rides the ones-row of
            # xs (w0x row E); L1 bias stays as K=2 matmuls.
            if "2" in cfg["phases"]:
              with tc.tile_pool(name="p2w", bufs=1) as pw, \
                 tc.tile_pool(name="p2x", bufs=3) as px, \
                 tc.tile_pool(name="p2t", bufs=2) as pt, \
                 tc.tile_pool(name="p2g0", bufs=1, space="PSUM") as pg0, \
                 tc.tile_pool(name="p2g1", bufs=1, space="PSUM") as pg1:
                fp8 = cfg["fp8"]
                w0x_sb = pw.tile([E + 1, G4], BF16)
                _dma(nc, w0x_sb, (w0xs_d if fp8 else w0x_d)[:, :])
                ones_sb = pw.tile([128, BL], BF16)
                nc.vector.memset(ones_sb, 1.0)
                for tb in range(2):
                    _dma(nc, _ap(xs_d, (tb * 128 * (E + 1) + E) * BL,
                                 [[(E + 1) * BL, 128], [1, BL]]), ones_sb)
                if fp8:
                    wh0_sb = pw.tile([128, 2, G4], FP8)
                    _dma(nc, wh0_sb, wh08_d[:, :, :])
                    wi1_sb = pw.tile([128, 2, G4], FP8)
                    _dma(nc, wi1_sb, wi18_d[:, :, :])
                    wh1_sb = pw.tile([128, 2, G4], BF16)
                    _dma(nc, wh1_sb, wh1s_d[:, :, :])
                    b1p_sb = pw.tile([2, 4, 128], BF16)
                    _dma(nc, b1p_sb, b1ps_d[:, :, :])
                    zw_sb = pw.tile([1, 128], BF16)
                    nc.vector.memset(zw_sb, 0.0)
                else:
                    wh0_sb = pw.tile([128, 2, G4], BF16)
                    _dma(nc, wh0_sb, wh0_d[:, :].rearrange("(c p) n -> p c n", p=128))
                    wi1_sb = pw.tile([128, 2, G4], BF16)
                    _dma(nc, wi1_sb, wi1_d[:, :].rearrange("(c p) n -> p c n", p=128))
                    wh1_sb = pw.tile([128, 2, G4], BF16)
                    _dma(nc, wh1_sb, wh1_d[:, :].rearrange("(c p) n -> p c n", p=128))
                    b1p_sb = pw.tile([2, 4, 128], BF16)
                    _dma(nc, b1p_sb, b1p_d[:, :, :])
                    if cfg["dstart"]:
                        zw_sb = pw.tile([1, 128], BF16)
                        nc.vector.memset(zw_sb, 0.0)
                    else:
                        b0p_sb = pw.tile([2, 4, 128], BF16)
                        _dma(nc, b0p_sb, b0p_d[:, :, :])
                sel2_sb = pw.tile([2, 512], BF16)
                _dma(nc, sel2_sb, sel2_d[:, :])

                h0T = pw.tile([128, 2, BL], FP8 if fp8 else BF16)
                h1T = pw.tile([128, 2, BL], BF16)
                c0 = pw.tile([128, 2, BL], cdt)
                c1 = pw.tile([128, 2, BL], cdt)
                nc.vector.memset(h0T, 0.0)
                nc.vector.memset(h1T, 0.0)
                nc.vector.memset(c0, 0.0)
                nc.vector.memset(c1, 0.0)

                inv_gs = 1.0 / GS if fp8 else 1.0

                def lstm_layer(g, sigp, hT, cT, h_scale):
                    # gate nonlinearity + cell update; gates in g ([128,8,BL] psum)
                    # pre-activations are scaled by GS in the fp8 path
                    sig = sigp.tile([128, 6, BL], BF16, tag="sig")
                    nc.scalar.activation(sig, g[:, 0:6, :], AF.Sigmoid, scale=inv_gs)
                    tg = sigp.tile([128, 2, BL], BF16, tag="tg")
                    nc.scalar.activation(tg, g[:, 6:8, :], AF.Tanh, scale=inv_gs)
                    t1 = sigp.tile([128, 2, BL], BF16, tag="t1")
                    nc.vector.tensor_mul(t1, sig[:, 0:2, :], tg)
                    nc.vector.tensor_mul(cT, sig[:, 2:4, :], cT)
                    nc.vector.tensor_add(cT, cT, t1)
                    tcc = sigp.tile([128, 2, BL], BF16, tag="tcc")
                    nc.scalar.activation(tcc, cT, AF.Tanh)
                    if h_scale == 1.0:
                        nc.vector.tensor_mul(hT, sig[:, 4:6, :], tcc)
                    else:
                        nc.vector.scalar_tensor_tensor(
                            out=hT, in0=sig[:, 4:6, :], scalar=h_scale, in1=tcc,
                            op0=ALU.mult, op1=ALU.mult)

                nsteps = cfg["nsteps"]
                for it in range(nsteps + 1):
                    do0 = it < nsteps
                    do1 = it > 0

                    # --- tensor queue: everything reads prev-iter state only
                    if do0:
                        kx = E + 1 if (fp8 or cfg["dstart"]) else E
                        xin = px.tile([kx, BL], BF16, tag="xin")
                        _dma(nc, xin, xs_d[it, 0:kx, :])
                        g0 = pg0.tile([128, 8, BL], FP32, tag="g0")
                        if fp8:
                            # N=1 zero-weight matmuls legally start each 2KB bank
                            for bk in range(4):
                                nc.tensor.matmul(g0[:, 2 * bk, 0:1], lhsT=zw_sb,
                                                 rhs=ones_sb[0:1, 0:1], start=True,
                                                 stop=False, skip_group_check=True)
                            for m in range(8):
                                nc.tensor.matmul(g0[:, m, :],
                                                 lhsT=w0x_sb[:, m * 128:(m + 1) * 128],
                                                 rhs=xin, start=False, stop=False,
                                                 skip_group_check=True)
                            for m in range(8):
                                nc.tensor.matmul(
                                    g0[:, m, :],
                                    lhsT=wh0_sb[:, :, m * 128:(m + 1) * 128],
                                    rhs=h0T, start=False, stop=True,
                                    perf_mode=mybir.MatmulPerfMode.DoubleRow,
                                    skip_group_check=True)
                        elif cfg["dstart"]:
                            # one N=1 zero-weight matmul starts (pending-zeros)
                            # each 2KB PSUM bank; bias rides the xs ones-row
                            for bk in range(4):
                                nc.tensor.matmul(g0[:, 2 * bk, 0:1], lhsT=zw_sb,
                                                 rhs=ones_sb[0:1, 0:1], start=True,
                                                 stop=False, skip_group_check=True)
                            # slot-major so sigmoid's slots 0:6 finish early
                            for m in range(8):
                                nc.tensor.matmul(g0[:, m, :],
                                                 lhsT=w0x_sb[:, m * 128:(m + 1) * 128],
                                                 rhs=xin, start=False,
                                                 stop=False, skip_group_check=True)
                                for kc in range(2):
                                    nc.tensor.matmul(g0[:, m, :],
                                                     lhsT=wh0_sb[:, kc, m * 128:(m + 1) * 128],
                                                     rhs=h0T[:, kc, :], start=False,
                                                     stop=(kc == 1), skip_group_check=True)
                        else:
                            for bk in range(4):
                                nc.tensor.matmul(g0[:, 2 * bk:2 * bk + 2, :],
                                                 lhsT=b0p_sb[:, bk, :],
                                                 rhs=sel2_sb, start=True, stop=False,
                                                 skip_group_check=True)
                            for m in range(8):
                                nc.tensor.matmul(g0[:, m, :],
                                                 lhsT=w0x_sb[0:E, m * 128:(m + 1) * 128],
                                                 rhs=xin, start=False,
                                                 stop=False, skip_group_check=True)
                            for kc in range(2):
                                for m in range(8):
                                    nc.tensor.matmul(g0[:, m, :],
                                                     lhsT=wh0_sb[:, kc, m * 128:(m + 1) * 128],
                                                     rhs=h0T[:, kc, :], start=False,
                                                     stop=(kc == 1), skip_group_check=True)
                    if do1:
                        g1 = pg1.tile([128, 8, BL], FP32, tag="g1")
                        for bk in range(4):
                            nc.tensor.matmul(g1[:, 2 * bk:2 * bk + 2, :],
                                             lhsT=b1p_sb[:, bk, :],
                                             rhs=sel2_sb, start=True, stop=False,
                                             skip_group_check=True)
                        # wi1 @ h0(it-1) first (h0T still holds it-1 here)
                        if fp8:
                            for m in range(8):
                                nc.tensor.matmul(
                                    g1[:, m, :],
                                    lhsT=wi1_sb[:, :, m * 128:(m + 1) * 128],
                                    rhs=h0T, start=False, stop=False,
                                    perf_mode=mybir.MatmulPerfMode.DoubleRow,
                                    skip_group_check=True)
                        else:
                            for m in range(8):
                                for kc in range(2):
                                    nc.tensor.matmul(g1[:, m, :],
                                                     lhsT=wi1_sb[:, kc, m * 128:(m + 1) * 128],
                                                     rhs=h0T[:, kc, :], start=False,
                                                     stop=False, skip_group_check=True)
                        # wh1 @ h1(it-2): h1T write for it-1 lands late, so last
                        for m in range(8):
                            for kc in range(2):
                                nc.tensor.matmul(g1[:, m, :],
                                                 lhsT=wh1_sb[:, kc, m * 128:(m + 1) * 128],
                                                 rhs=h1T[:, kc, :], start=False,
                                                 stop=(kc == 1), skip_group_check=True)

                    # --- scalar/vector queues (overlap next iter's matmuls)
                    if do0:
                        if cfg["debug"] and it == 0:
                            g0cp = pt.tile([128, 8, BL], FP32, tag="g0cp")
                            nc.vector.tensor_copy(g0cp, g0)
                            _dma(nc, dbg_g0_d[:, :, :], g0cp)
                        lstm_layer(g0, pt, h0T, c0, HS if fp8 else 1.0)
                        if cfg["debug"]:
                            h0cp = pt.tile([128, 2, BL], FP32, tag="h0cp")
                            nc.vector.tensor_copy(h0cp, h0T)
                            _dma(nc, dbg_h0_d[it, :, :, :].rearrange("h p b -> p h b"),
                                 h0cp)
                    if do1:
                        lstm_layer(g1, pt, h1T, c1, 1.0)
                        if cfg["debug"]:
                            _dma(nc, hs_d[it - 1, :, :, :].rearrange("h p b -> p h b"),
                                 h1T)
                        else:
                            _dma(nc, hs_ds[(it - 1) // TCH][(it - 1) % TCH, :, :, :]
                                 .rearrange("h p b -> p h b"), h1T)

            # =========================== P3: head ===========================
            if "3" in cfg["phases"]:
              with tc.tile_pool(name="p3in", bufs=1) as pin, \
                 tc.tile_pool(name="p3t", bufs=4) as ptmp, \
                 tc.tile_pool(name="p3ps", bufs=6, space="PSUM") as pps:
                hwt_sb = pin.tile([128, 2, 20], BF16)
                _dma(nc, hwt_sb, hwt_d[:, :].rearrange("(c p) k -> p c k", p=128))
                hb16_sb = pin.tile([1, 2 * TCH * 20], BF16)
                _dma(nc, hb16_sb, hb16_d[:, :])
                ones1b = pin.tile([1, 128], BF16)
                nc.vector.memset(ones1b, 1.0)
                iotak_sb = pin.tile([128, 20], BF16)
                _dma(nc, iotak_sb, iotak_d[:, :])
                xb_sb = pin.tile([128, 2, D], BF16)
                _dma(nc, xb_sb, xb_d[:, :, :])
                # se values for all (hf, t): ln deferred to one batched pass so
                # the Exp/Ln activation tables are not reloaded per chunk
                se_all = pin.tile([128, 2, D], FP32)
                gth_all = pin.tile([128, 2, NCH], FP32)

                # one-hot(x) masks for ALL (hf, t, k) in a single broadcast
                # compare, off the per-chunk critical chain
                mask_all = pin.tile([128, 2, D, 20], BF16)
                xbc = bass.AP(tensor=xb_sb.tensor, offset=xb_sb.offset,
                              ap=[list(d) for d in xb_sb[:, :, :].ap] + [[0, 20]])
                iob = bass.AP(
                    tensor=iotak_sb.tensor, offset=iotak_sb.offset,
                    ap=[list(iotak_sb[:, :].ap[0]), [0, 2], [0, D], [1, 20]])
                nc.vector.tensor_tensor(mask_all, xbc, iob, ALU.is_equal)

                for ci in range(NCH):
                    t0 = ci * TCH
                    hsb = ptmp.tile([128, 2, TCH, BL], BF16, tag="hsb")
                    for kc in range(2):
                        # kc=0 on gpsimd so it prefetches during P2 (the sync
                        # queue is paced by P2's per-iteration stores)
                        eng = nc.gpsimd if kc == 0 else nc.sync
                        if cfg["debug"]:
                            src_ap = _ap(hs_d, (t0 * 2 + kc) * 128 * BL,
                                         [[BL, 128], [2 * 128 * BL, TCH], [1, BL]])
                        else:
                            src_ap = _ap(hs_ds[ci], kc * 128 * BL,
                                         [[BL, 128], [2 * 128 * BL, TCH], [1, BL]])
                        eng.dma_start(out=hsb[:, kc, :, :], in_=src_ap)
                    # one PSUM bank holds logits for both batch halves
                    ps_lg = pps.tile([128, 2, TCH, 20], FP32, tag="ps_lg")
                    nc.tensor.matmul(ps_lg[:, :, :, :], lhsT=ones1b, rhs=hb16_sb,
                                     start=True, stop=False, skip_group_check=True)
                    for hf in range(2):
                        for tt in range(TCH):
                            for kc in range(2):
                                nc.tensor.matmul(
                                    ps_lg[:, hf, tt, :],
                                    lhsT=hsb[:, kc, tt, hf * 128:(hf + 1) * 128],
                                    rhs=hwt_sb[:, kc, :],
                                    start=False, stop=(kc == 1), skip_group_check=True)
                    elg = ptmp.tile([128, 2, TCH, 20], FP32, tag="elg")
                    nc.scalar.activation(elg, ps_lg, AF.Exp)
                    nc.vector.tensor_reduce(se_all[:, :, t0:t0 + TCH], elg,
                                            axis=mybir.AxisListType.X, op=ALU.add)
                    # fused multiply+reduce of precomputed one-hot masks
                    # against the logits per half
                    scr = ptmp.tile([128, TCH, 20], FP32, tag="scr")
                    for hf in range(2):
                        nc.vector.scalar_tensor_tensor(
                            out=scr, in0=ps_lg[:, hf], scalar=1.0,
                            in1=mask_all[:, hf, t0:t0 + TCH, :],
                            op0=ALU.bypass, op1=ALU.mult,
                            accum_out=gth_all[:, hf, ci:ci + 1])

                lse2 = pin.tile([128, 2], FP32)
                for hf in range(2):
                    lnall = pin.tile([128, D], FP32, name=f"lnall_{hf}")
                    nc.scalar.activation(lnall, se_all[:, hf, :], AF.Ln,
                                         accum_out=lse2[:, hf:hf + 1])
                gth2 = pin.tile([128, 2], FP32)
                nc.vector.tensor_reduce(gth2, gth_all[:, :, :],
                                        axis=mybir.AxisListType.X, op=ALU.add)
                res = pin.tile([128, 2], FP32)
                nc.vector.tensor_sub(res, gth2, lse2)
                _dma(nc, _ap(logq_d, 0, [[1, 128], [128, 2]]), res)

    nc.compile()
    return nc


def _prep(inputs, cfg):
    f32 = np.float32
    emb = np.asarray(inputs["emb"], f32)
    perm = np.r_[0:2 * H, 3 * H:4 * H, 2 * H:3 * H]   # torch i,f,g,o -> i,f,o,g

    def pair_bias(b):
        # b[G4] -> [2, 4, 128]: b0p[j, bk, p] = b[(2bk+j)*128+p]
        return b.reshape(4, 2, 128).transpose(1, 0, 2).copy()

    wih0 = np.asarray(inputs["wih0"], f32)
    whh0 = np.asarray(inputs["whh0"], f32)
    wih1 = np.asarray(inputs["wih1"], f32)
    whh1 = np.asarray(inputs["whh1"], f32)
    b0 = (np.asarray(inputs["bih0"], f32) + np.asarray(inputs["bhh0"], f32))[perm]
    b1 = (np.asarray(inputs["bih1"], f32) + np.asarray(inputs["bhh1"], f32))[perm]

    emb2 = np.zeros((40, 128), f32)
    emb2[:20, :64] = emb
    emb2[20:, 64:] = emb
    sel2 = np.zeros((2, 512), f32)
    sel2[0, :256] = 1.0
    sel2[1, 256:] = 1.0
    gb2 = np.asarray(inputs["gb2"], f32)
    bb2 = np.asarray(inputs["bb2"], f32)

    shared = {
        "gw1t": np.ascontiguousarray(np.asarray(inputs["gw1"], f32).T),
        "bw1t": np.ascontiguousarray(np.asarray(inputs["bw1"], f32).T),
        "gb1": np.asarray(inputs["gb1"], f32),
        "bb1": np.asarray(inputs["bb1"], f32),
        "glnw": np.asarray(inputs["gln_w"], f32),
        "glnb": np.asarray(inputs["gln_b"], f32),
        "blnw": np.asarray(inputs["bln_w"], f32),
        "blnb": np.asarray(inputs["bln_b"], f32),
        "gw2t": np.ascontiguousarray(np.asarray(inputs["gw2"], f32).T),
        "bw2t": np.ascontiguousarray(np.asarray(inputs["bw2"], f32).T),
        "gb2d": np.concatenate([gb2, gb2])[:, None].copy(),
        "bb2d": np.concatenate([bb2, bb2])[:, None].copy(),
        "u0wt": np.ascontiguousarray(np.asarray(inputs["u0_w"], f32).T),
        "u0b": np.asarray(inputs["u0_b"], f32)[:, None].copy(),
        "ident": np.eye(128, dtype=f32),
        "emb2": emb2.astype(BF),
        "iota2": (np.arange(40, dtype=f32) % 20)[:, None].copy(),
        "w0x": np.ascontiguousarray(np.vstack([wih0.T[:, perm], b0[None, :]])).astype(BF),
        "wh0": np.ascontiguousarray(whh0.T[:, perm]).astype(BF),
        "wi1": np.ascontiguousarray(wih1.T[:, perm]).astype(BF),
        "wh1": np.ascontiguousarray(whh1.T[:, perm]).astype(BF),
        "b0p": pair_bias(b0).astype(BF),
        "b1p": pair_bias(b1).astype(BF),
        # fp8 path: gate pre-activations carry a GS=256 scale (wh0/wi1 are
        # fp8 x16 and h0 is stored as fp8 x16; bf16 contributions are x256)
        "w0xs": (np.vstack([wih0.T[:, perm], b0[None, :]]) * GS).astype(BF),
        "wh08": np.ascontiguousarray(
            (whh0.T[:, perm] * HS).reshape(2, 128, G4).transpose(1, 0, 2)).astype(F8),
        "wi18": np.ascontiguousarray(
            (wih1.T[:, perm] * HS).reshape(2, 128, G4).transpose(1, 0, 2)).astype(F8),
        "wh1s": np.ascontiguousarray(
            (whh1.T[:, perm] * GS).reshape(2, 128, G4).transpose(1, 0, 2)).astype(BF),
        "b1ps": (pair_bias(b1) * GS).astype(BF),
        "sel2": sel2.astype(BF),
        "hwt": np.ascontiguousarray(np.asarray(inputs["head_w"], f32).T).astype(BF),
        "hb8": np.tile(np.asarray(inputs["head_b"], f32), TCH)[None, :].astype(BF),
        "hb16": np.tile(np.asarray(inputs["head_b"], f32), 2 * TCH)[None, :].astype(BF),
        "iotak": np.tile(np.arange(20, dtype=f32), (128, 1)).astype(BF),
    }

    U = np.asarray(inputs["U"], f32)
    X = np.asarray(inputs["X"])
    in_maps = []
    for c in range(N_CORES):
        Us = U[c * BL:(c + 1) * BL]
        Xs = X[c * BL:(c + 1) * BL]
        m = dict(shared)
        m["ut"] = np.ascontiguousarray(Us.T)
        m["xt"] = np.ascontiguousarray(Xs.T.astype(f32)).astype(BF)
        m["xb"] = np.ascontiguousarray(
            Xs.astype(f32).reshape(2, 128, D).transpose(1, 0, 2)).astype(BF)
        in_maps.append(m)
    return in_maps


def get_nc(cfg=None):
    cfg = dict(CFG, **(cfg or {}))
    key = tuple(sorted(cfg.items()))
    if key not in _CACHE:
        _CACHE[key] = build_nc(cfg)
    return _CACHE[key]


def run(inputs, cfg=None, **run_kwargs):
    cfg = dict(CFG, **(cfg or {}))
    nc = get_nc(cfg)
    in_maps = _prep(inputs, cfg)
    res = run_bass_kernel_spmd(nc, in_maps, core_ids=list(range(N_CORES)), **run_kwargs)
    return res


def kernel(**inputs):
    res = run(inputs)
    out = np.concatenate([res.results[c]["logq"] for c in range(N_CORES)])
    return out.astype(np.float32)



# revision 41
# speedup vs baseline: 1.0348x; 1.0302x over previous
"""CondLSTMProposal Trainium2 kernel.

Data-parallel over batch: 8 cores x 256 batch rows each. Everything on-chip
runs in transposed [feature, batch] layout so the LSTM recurrence needs no
transposes. Phases:
  P0  FiLM conditioning (fp32): gamT/betT/e0T per core.
  P1  teacher-forced inputs: one-hot(X) @ emb (pair-packed, bf16 matmul),
      FiLM-modulate, store xsT[t] to DRAM (bf16).
  P2  2-layer LSTM, software-pipelined: iteration `it` runs layer0 step `it`
      and layer1 step `it-1`, so every matmul of an iteration reads only
      previous-iteration state and the PE issue stream never stalls (keeps
      the HAM clock-gate warm at 8/8 - this alone is worth ~1.4x). Layer-0
      PSUM banks are started by N=1 zero-weight matmuls with the bias riding
      the xs ones-row (a start=True matmul marks its whole 2KB PSUM bank
      pending-zero, so each bank must be started exactly once, bank-wide).
      bf16 matmuls into fp32 PSUM, bf16 activations, fp32 cell state.
  P3  head: logits in [batch, t, k] layout, one PSUM bank per 8-step chunk
      covering both batch halves; exp per chunk, ln deferred to one batched
      pass (avoids per-chunk Exp/Ln activation-table reloads); one-hot
      gather via a 0-stride broadcast is_equal + scalar_tensor_tensor with
      accumulate; hs split into per-chunk DRAM tensors so chunk loads
      prefetch during P2.
"""

import sys

sys.path.insert(0, "/opt/trn_rl_repo")

import numpy as np
import ml_dtypes

import concourse.bass as bass
import concourse.bacc as bacc
import concourse.tile as tile
from concourse import mybir
from concourse.bass_utils import run_bass_kernel_spmd


def _install_ntff_hook_shim():
    """Provide antenv.axon_hooks (absent from this image) so trace=True works."""
    import types
    if "antenv.axon_hooks" in sys.modules:
        return
    mod = types.ModuleType("antenv.axon_hooks")
    state = {"hook": None}

    def set_axon_ntff_profile_hook(hook):
        state["hook"] = hook

    def get_axon_ntff_profile_hook():
        if state["hook"] is None:
            try:
                from trn_agent_boot.trn_boot import _ntff_profile_via_ctypes
                state["hook"] = _ntff_profile_via_ctypes("/opt/axon/libaxon_pjrt.so")
            except Exception:
                state["hook"] = None
        return state["hook"]

    mod.set_axon_ntff_profile_hook = set_axon_ntff_profile_hook
    mod.get_axon_ntff_profile_hook = get_axon_ntff_profile_hook
    sys.modules["antenv.axon_hooks"] = mod
    try:
        import antenv
        antenv.axon_hooks = mod
    except ImportError:
        pass


_install_ntff_hook_shim()

FP32 = mybir.dt.float32
BF16 = mybir.dt.bfloat16
FP8 = mybir.dt.float8e4
AF = mybir.ActivationFunctionType
ALU = mybir.AluOpType
BF = ml_dtypes.bfloat16
F8 = ml_dtypes.float8_e4m3fn
GS = 256.0                 # gate pre-activation scale (wh/wi fp8 x16, h0 fp8 x16)
HS = 16.0                  # h0 fp8 scale

B, D, K, UD, E, H = 2048, 256, 20, 512, 64, 256
HID = 512
N_CORES = 8
BL = B // N_CORES          # 256 per-core batch
G4 = 4 * H                 # 1024 gate rows
LN_EPS = 1e-5
TCH = 8                    # timesteps per P1/P3 chunk
NCH = D // TCH             # 32 chunks

CFG = {
    "debug": False,        # expose xs/hs/gam/bet as outputs
    "c_fp32": True,        # keep LSTM cell state in fp32
    "phases": "0123",     # which phases to emit (bisection aid)
    "nsteps": D,           # LSTM steps to emit
    "p1_level": 4,         # P1 sub-bisect: 1=dma+mask 2=+mm 3=+mod 4=+out
    "onesrow": False,      # (non-fp8 path) L0 bias via ones-row is WRONG with
                           # per-slot start=True: start marks the whole 2KB PSUM
                           # bank pending-zero, clobbering the sibling half-bank.
                           # fp8 path fixes this with N=1 zero-weight bank starts.
    "fp8": False,          # fp8 DoubleRow was a loss: DR disables FWL, matmuls
                           # got slower (301ns vs 2x107ns) and HAM went cold
    "dstart": True,        # g0: N=1 zero-weight bank starts + ones-row bias
                           # (replaces 4 N=512 bias matmuls per iteration)
}

_CACHE = {}


def _dma(nc, out, in_):
    nc.sync.dma_start(out=out, in_=in_)


def _bcast_dma(nc, out, in_ap):
    # partition-broadcast / fancy-AP DMAs go through gpsimd (SWDGE)
    nc.gpsimd.dma_start(out=out, in_=in_ap)


def _ap(handle, offset, dims):
    base = handle[tuple(slice(None) for _ in handle.shape)]
    return bass.AP(tensor=base.tensor, offset=offset, ap=[list(d) for d in dims])


def build_nc(cfg):
    nc = bacc.Bacc("TRN2")

    # ---- per-core inputs
    ut_d = nc.dram_tensor("ut", [UD, BL], FP32, kind="ExternalInput")
    xt_d = nc.dram_tensor("xt", [D, BL], BF16, kind="ExternalInput")
    xb_d = nc.dram_tensor("xb", [128, 2, D], BF16, kind="ExternalInput")

    # ---- replicated weights / constants
    gw1t_d = nc.dram_tensor("gw1t", [UD, HID], FP32, kind="ExternalInput")
    bw1t_d = nc.dram_tensor("bw1t", [UD, HID], FP32, kind="ExternalInput")
    gb1_d = nc.dram_tensor("gb1", [HID], FP32, kind="ExternalInput")
    bb1_d = nc.dram_tensor("bb1", [HID], FP32, kind="ExternalInput")
    glnw_d = nc.dram_tensor("glnw", [HID], FP32, kind="ExternalInput")
    glnb_d = nc.dram_tensor("glnb", [HID], FP32, kind="ExternalInput")
    blnw_d = nc.dram_tensor("blnw", [HID], FP32, kind="ExternalInput")
    blnb_d = nc.dram_tensor("blnb", [HID], FP32, kind="ExternalInput")
    gw2t_d = nc.dram_tensor("gw2t", [HID, E], FP32, kind="ExternalInput")
    bw2t_d = nc.dram_tensor("bw2t", [HID, E], FP32, kind="ExternalInput")
    gb2d_d = nc.dram_tensor("gb2d", [128, 1], FP32, kind="ExternalInput")
    bb2d_d = nc.dram_tensor("bb2d", [128, 1], FP32, kind="ExternalInput")
    u0wt_d = nc.dram_tensor("u0wt", [UD, E], FP32, kind="ExternalInput")
    u0b_d = nc.dram_tensor("u0b", [E, 1], FP32, kind="ExternalInput")
    ident_d = nc.dram_tensor("ident", [128, 128], FP32, kind="ExternalInput")

    emb2_d = nc.dram_tensor("emb2", [40, 128], BF16, kind="ExternalInput")
    iota2_d = nc.dram_tensor("iota2", [40, 1], FP32, kind="ExternalInput")

    w0x_d = nc.dram_tensor("w0x", [E + 1, G4], BF16, kind="ExternalInput")
    wh0_d = nc.dram_tensor("wh0", [H, G4], BF16, kind="ExternalInput")
    wi1_d = nc.dram_tensor("wi1", [H, G4], BF16, kind="ExternalInput")
    wh1_d = nc.dram_tensor("wh1", [H, G4], BF16, kind="ExternalInput")
    w0xs_d = nc.dram_tensor("w0xs", [E + 1, G4], BF16, kind="ExternalInput")
    wh08_d = nc.dram_tensor("wh08", [128, 2, G4], FP8, kind="ExternalInput")
    wi18_d = nc.dram_tensor("wi18", [128, 2, G4], FP8, kind="ExternalInput")
    wh1s_d = nc.dram_tensor("wh1s", [128, 2, G4], BF16, kind="ExternalInput")
    b1ps_d = nc.dram_tensor("b1ps", [2, 4, 128], BF16, kind="ExternalInput")
    b0p_d = nc.dram_tensor("b0p", [2, 4, 128], BF16, kind="ExternalInput")
    b1p_d = nc.dram_tensor("b1p", [2, 4, 128], BF16, kind="ExternalInput")
    sel2_d = nc.dram_tensor("sel2", [2, 512], BF16, kind="ExternalInput")

    hwt_d = nc.dram_tensor("hwt", [H, 20], BF16, kind="ExternalInput")
    hb8_d = nc.dram_tensor("hb8", [1, TCH * 20], BF16, kind="ExternalInput")
    hb16_d = nc.dram_tensor("hb16", [1, 2 * TCH * 20], BF16, kind="ExternalInput")
    iotak_d = nc.dram_tensor("iotak", [128, 20], BF16, kind="ExternalInput")

    # ---- outputs / scratch
    logq_d = nc.dram_tensor("logq", [BL], FP32, kind="ExternalOutput")
    sk = "ExternalOutput" if cfg["debug"] else "Internal"
    xs_d = nc.dram_tensor("xs", [D + 1, E + 1, BL], BF16, kind=sk)
    hs_d = nc.dram_tensor("hs", [D, 2, 128, BL], BF16, kind=sk)
    # per-chunk hs tensors: P3's chunk loads depend only on that chunk's 8
    # stores (DRAM deps are tracked per-tensor), so they prefetch during P2
    hs_ds = [nc.dram_tensor(f"hsc{ci}", [TCH, 2, 128, BL], BF16, kind="Internal")
             for ci in range(NCH)]
    if cfg["debug"]:
        dbg_gam_d = nc.dram_tensor("dbg_gam", [128, 4, BL], FP32, kind="ExternalOutput")
        dbg_bet_d = nc.dram_tensor("dbg_bet", [128, 4, BL], FP32, kind="ExternalOutput")
        dbg_g0_d = nc.dram_tensor("dbg_g0", [128, 8, BL], FP32, kind="ExternalOutput")
        dbg_h0_d = nc.dram_tensor("dbg_h0", [D, 2, 128, BL], FP32, kind="ExternalOutput")

    cdt = FP32 if cfg["c_fp32"] else BF16

    with tile.TileContext(nc) as tc:
        with tc.tile_pool(name="glob", bufs=1) as glob:
            # persistent across phases
            gp4 = glob.tile([128, 4, BL], FP32)   # (1+gam) doubled over partitions, x4 pair slots
            bt4 = glob.tile([128, 4, BL], FP32)

            # =========================== P0: FiLM ===========================
            if "0" in cfg["phases"]:
              with tc.tile_pool(name="p0in", bufs=1) as pin, \
                 tc.tile_pool(name="p0t", bufs=2) as ptmp, \
                 tc.tile_pool(name="p0ps", bufs=2, space="PSUM") as pps:
                ut_sb = pin.tile([128, 4, BL], FP32)
                _dma(nc, ut_sb, ut_d[:, :].rearrange("(c p) b -> p c b", p=128))
                ident_sb = pin.tile([128, 128], FP32)
                _dma(nc, ident_sb, ident_d[:, :])
                eps_sb = pin.tile([128, 1], FP32)
                nc.vector.memset(eps_sb, LN_EPS)

                branches = [
                    (gw1t_d, gb1_d, glnw_d, glnb_d, gw2t_d, gb2d_d, gp4),
                    (bw1t_d, bb1_d, blnw_d, blnb_d, bw2t_d, bb2d_d, bt4),
                ]
                for br, (w1d, b1d, lnwd, lnbd, w2d, b2dd, dst) in enumerate(branches):
                    w1_sb = pin.tile([128, 4, HID], FP32, name=f"w1_{br}")
                    _dma(nc, w1_sb, w1d[:, :].rearrange("(c p) n -> p c n", p=128))
                    b1b_sb = pin.tile([128, HID], FP32, name=f"b1b_{br}")
                    _bcast_dma(nc, b1b_sb, _ap(b1d, 0, [[0, 128], [1, HID]]))
                    lnw_sb = pin.tile([128, HID], FP32, name=f"lnw_{br}")
                    _bcast_dma(nc, lnw_sb, _ap(lnwd, 0, [[0, 128], [1, HID]]))
                    lnb_sb = pin.tile([128, HID], FP32, name=f"lnb_{br}")
                    _bcast_dma(nc, lnb_sb, _ap(lnbd, 0, [[0, 128], [1, HID]]))
                    w2_sb = pin.tile([128, 4, E], FP32, name=f"w2_{br}")
                    _dma(nc, w2_sb, w2d[:, :].rearrange("(c p) e -> p c e", p=128))
                    b2_sb = pin.tile([128, 1], FP32, name=f"b2_{br}")
                    _dma(nc, b2_sb, b2dd[:, :])

                    sT = pin.tile([128, 4, BL], FP32, name=f"sT_{br}")
                    for mb in range(2):
                        ps_h = pps.tile([128, HID], FP32, tag="ps_h")
                        for c in range(4):
                            nc.tensor.matmul(
                                ps_h, lhsT=ut_sb[:, c, mb * 128:(mb + 1) * 128],
                                rhs=w1_sb[:, c, :], start=(c == 0), stop=(c == 3))
                        h_sb = ptmp.tile([128, HID], FP32, tag="h_sb")
                        nc.vector.tensor_add(h_sb, ps_h, b1b_sb)
                        stats = ptmp.tile([128, 6], FP32, tag="stats")
                        nc.vector.bn_stats(out=stats, in_=h_sb)
                        mv = ptmp.tile([128, 2], FP32, tag="mv")
                        nc.vector.bn_aggr(out=mv, in_=stats)
                        std = ptmp.tile([128, 1], FP32, tag="std")
                        nc.scalar.activation(std, mv[:, 1:2], AF.Sqrt, bias=eps_sb)
                        rstd = ptmp.tile([128, 1], FP32, tag="rstd")
                        nc.vector.reciprocal(rstd, std)
                        nc.vector.tensor_scalar(
                            out=h_sb, in0=h_sb, scalar1=mv[:, 0:1], scalar2=rstd,
                            op0=ALU.subtract, op1=ALU.mult)
                        nc.vector.tensor_mul(h_sb, h_sb, lnw_sb)
                        nc.vector.tensor_add(h_sb, h_sb, lnb_sb)
                        s_sb = ptmp.tile([128, HID], FP32, tag="s_sb")
                        nc.scalar.activation(s_sb, h_sb, AF.Sigmoid)
                        nc.vector.tensor_mul(s_sb, s_sb, h_sb)
                        for c in range(4):
                            ps_t = pps.tile([128, 128], FP32, tag="ps_t")
                            nc.tensor.transpose(ps_t, s_sb[:, c * 128:(c + 1) * 128], ident_sb)
                            nc.scalar.copy(sT[:, c, mb * 128:(mb + 1) * 128], ps_t)

                    ps_o = pps.tile([128, BL], FP32, tag="ps_o")
                    for hf in range(2):
                        for c in range(4):
                            nc.tensor.matmul(
                                ps_o[hf * 64:(hf + 1) * 64, :],
                                lhsT=w2_sb[:, c, :], rhs=sT[:, c, :],
                                start=(c == 0), stop=(c == 3))
                    for j in range(4):
                        if br == 0:
                            nc.vector.tensor_scalar(
                                out=dst[:, j, :], in0=ps_o, scalar1=b2_sb, scalar2=1.0,
                                op0=ALU.add, op1=ALU.add)
                        else:
                            nc.vector.tensor_scalar_add(out=dst[:, j, :], in0=ps_o, scalar1=b2_sb)

                    if cfg["debug"]:
                        dd = dbg_gam_d if br == 0 else dbg_bet_d
                        _dma(nc, dd[:, :, :], dst[:, :, :])

                # e0T -> xs[0]
                u0w_sb = pin.tile([128, 4, E], FP32)
                _dma(nc, u0w_sb, u0wt_d[:, :].rearrange("(c p) e -> p c e", p=128))
                u0b_sb = pin.tile([E, 1], FP32)
                _dma(nc, u0b_sb, u0b_d[:, :])
                ps_e0 = pps.tile([E, BL], FP32, tag="ps_e0")
                for c in range(4):
                    nc.tensor.matmul(ps_e0, lhsT=u0w_sb[:, c, :], rhs=ut_sb[:, c, :],
                                     start=(c == 0), stop=(c == 3))
                e0bf = ptmp.tile([E, BL], BF16, tag="e0bf")
                nc.vector.tensor_scalar_add(out=e0bf, in0=ps_e0, scalar1=u0b_sb)
                _dma(nc, xs_d[0, 0:E, :], e0bf)

            # =========================== P1: xs =============================
            if "1" in cfg["phases"]:
              with tc.tile_pool(name="p1in", bufs=1) as pin, \
                 tc.tile_pool(name="p1t", bufs=3) as ptmp, \
                 tc.tile_pool(name="p1ps", bufs=2, space="PSUM") as pps:
                emb2_sb = pin.tile([40, 128], BF16)
                _dma(nc, emb2_sb, emb2_d[:, :])
                iota2_sb = pin.tile([40, 1], FP32)
                _dma(nc, iota2_sb, iota2_d[:, :])
                for ci in range(NCH):
                    t0 = ci * TCH
                    xb4 = ptmp.tile([40, 4, BL], BF16, tag="xb4")
                    for j in range(2):
                        _bcast_dma(nc, xb4[j * 20:(j + 1) * 20, :, :],
                                   _ap(xt_d, (t0 + j) * BL,
                                       [[0, 20], [2 * BL, 4], [1, BL]]))
                    mask = ptmp.tile([40, 4, BL], BF16, tag="m")
                    nc.vector.tensor_scalar(out=mask, in0=xb4, scalar1=iota2_sb,
                                            scalar2=None, op0=ALU.is_equal)
                    if cfg["p1_level"] < 2:
                        continue
                    ps_sel = pps.tile([128, 4, BL], FP32, tag="ps_sel")
                    for i in range(4):
                        nc.tensor.matmul(ps_sel[:, i, :], lhsT=emb2_sb, rhs=mask[:, i, :],
                                         start=True, stop=True)
                    if cfg["p1_level"] < 3:
                        xsb = ptmp.tile([128, 4, BL], BF16, tag="xsb")
                        nc.vector.tensor_copy(xsb, ps_sel)
                    else:
                        tmp = ptmp.tile([128, 4, BL], FP32, tag="tmp")
                        nc.vector.tensor_mul(tmp, ps_sel, gp4)
                        xsb = ptmp.tile([128, 4, BL], BF16, tag="xsb")
                        nc.vector.tensor_add(xsb, tmp, bt4)
                    if cfg["p1_level"] < 4:
                        continue
                    _dma(nc, _ap(xs_d, (t0 + 1) * (E + 1) * BL,
                                 [[BL, E], [2 * (E + 1) * BL, 4], [1, BL]]), xsb[0:E, :, :])
                    _dma(nc, _ap(xs_d, (t0 + 2) * (E + 1) * BL,
                                 [[BL, E], [2 * (E + 1) * BL, 4], [1, BL]]), xsb[E:128, :, :])

            # =========================== P2: LSTM ===========================
            # Software-pipelined: iteration it runs L0 step `it` and L1 step
            # `it-1`, so every matmul in an iteration reads only state written
            # in earlier iterations and the PE never stalls mid-iteration
            # (keeps the HAM clock-gate warm). L0 bias rides the ones-row of
            # xs (w0x row E); L1 bias stays as K=2 matmuls.
            if "2" in cfg["phases"]:
              with tc.tile_pool(name="p2w", bufs=1) as pw, \
                 tc.tile_pool(name="p2x", bufs=3) as px, \
                 tc.tile_pool(name="p2t", bufs=2) as pt, \
                 tc.tile_pool(name="p2g0", bufs=1, space="PSUM") as pg0, \
                 tc.tile_pool(name="p2g1", bufs=1, space="PSUM") as pg1:
                fp8 = cfg["fp8"]
                w0x_sb = pw.tile([E + 1, G4], BF16)
                _dma(nc, w0x_sb, (w0xs_d if fp8 else w0x_d)[:, :])
                ones_sb = pw.tile([128, BL], BF16)
                nc.vector.memset(ones_sb, 1.0)
                for tb in range(2):
                    _dma(nc, _ap(xs_d, (tb * 128 * (E + 1) + E) * BL,
                                 [[(E + 1) * BL, 128], [1, BL]]), ones_sb)
                if fp8:
                    wh0_sb = pw.tile([128, 2, G4], FP8)
                    _dma(nc, wh0_sb, wh08_d[:, :, :])
                    wi1_sb = pw.tile([128, 2, G4], FP8)
                    _dma(nc, wi1_sb, wi18_d[:, :, :])
                    wh1_sb = pw.tile([128, 2, G4], BF16)
                    _dma(nc, wh1_sb, wh1s_d[:, :, :])
                    b1p_sb = pw.tile([2, 4, 128], BF16)
                    _dma(nc, b1p_sb, b1ps_d[:, :, :])
                    zw_sb = pw.tile([1, 128], BF16)
                    nc.vector.memset(zw_sb, 0.0)
                else:
                    wh0_sb = pw.tile([128, 2, G4], BF16)
                    _dma(nc, wh0_sb, wh0_d[:, :].rearrange("(c p) n -> p c n", p=128))
                    wi1_sb = pw.tile([128, 2, G4], BF16)
                    _dma(nc, wi1_sb, wi1_d[:, :].rearrange("(c p) n -> p c n", p=128))
                    wh1_sb = pw.tile([128, 2, G4], BF16)
                    _dma(nc, wh1_sb, wh1_d[:, :].rearrange("(c p) n -> p c n", p=128))
                    b1p_sb = pw.tile([2, 4, 128], BF16)
                    _dma(nc, b1p_sb, b1p_d[:, :, :])
                    if cfg["dstart"]:
                        # zero-padded xin (3 rotating slots): the even-slot
                        # x-matmul streams [xin | zeros] as a bank-wide N=512
                        # start, writing W@x into its slot and 0 into the odd
                        # sibling slot in one instruction
                        xin2 = pw.tile([E + 1, 3, 2 * BL], BF16)
                        nc.vector.memset(xin2, 0.0)
                    else:
                        b0p_sb = pw.tile([2, 4, 128], BF16)
                        _dma(nc, b0p_sb, b0p_d[:, :, :])
                sel2_sb = pw.tile([2, 512], BF16)
                _dma(nc, sel2_sb, sel2_d[:, :])

                h0T = pw.tile([128, 2, BL], FP8 if fp8 else BF16)
                h1T = pw.tile([128, 2, BL], BF16)
                c0 = pw.tile([128, 2, BL], cdt)
                c1 = pw.tile([128, 2, BL], cdt)
                nc.vector.memset(h0T, 0.0)
                nc.vector.memset(h1T, 0.0)
                nc.vector.memset(c0, 0.0)
                nc.vector.memset(c1, 0.0)

                inv_gs = 1.0 / GS if fp8 else 1.0

                def lstm_layer(g, sigp, hT, cT, h_scale):
                    # gate nonlinearity + cell update; gates in g ([128,8,BL] psum)
                    # pre-activations are scaled by GS in the fp8 path
                    sig = sigp.tile([128, 6, BL], BF16, tag="sig")
                    nc.scalar.activation(sig, g[:, 0:6, :], AF.Sigmoid, scale=inv_gs)
                    tg = sigp.tile([128, 2, BL], BF16, tag="tg")
                    nc.scalar.activation(tg, g[:, 6:8, :], AF.Tanh, scale=inv_gs)
                    t1 = sigp.tile([128, 2, BL], BF16, tag="t1")
                    nc.vector.tensor_mul(t1, sig[:, 0:2, :], tg)
                    nc.vector.tensor_mul(cT, sig[:, 2:4, :], cT)
                    nc.vector.tensor_add(cT, cT, t1)
                    tcc = sigp.tile([128, 2, BL], BF16, tag="tcc")
                    nc.scalar.activation(tcc, cT, AF.Tanh)
                    if h_scale == 1.0:
                        nc.vector.tensor_mul(hT, sig[:, 4:6, :], tcc)
                    else:
                        nc.vector.scalar_tensor_tensor(
                            out=hT, in0=sig[:, 4:6, :], scalar=h_scale, in1=tcc,
                            op0=ALU.mult, op1=ALU.mult)

                nsteps = cfg["nsteps"]
                for it in range(nsteps + 1):
                    do0 = it < nsteps
                    do1 = it > 0

                    # --- tensor queue: everything reads prev-iter state only
                    if do0:
                        if cfg["dstart"] and not fp8:
                            sl = it % 3
                            _dma(nc, xin2[:, sl, 0:BL], xs_d[it, 0:E + 1, :])
                        else:
                            kx = E + 1 if fp8 else E
                            xin = px.tile([kx, BL], BF16, tag="xin")
                            _dma(nc, xin, xs_d[it, 0:kx, :])
                        g0 = pg0.tile([128, 8, BL], FP32, tag="g0")
                        if fp8:
                            # N=1 zero-weight matmuls legally start each 2KB bank
                            for bk in range(4):
                                nc.tensor.matmul(g0[:, 2 * bk, 0:1], lhsT=zw_sb,
                                                 rhs=ones_sb[0:1, 0:1], start=True,
                                                 stop=False, skip_group_check=True)
                            for m in range(8):
                                nc.tensor.matmul(g0[:, m, :],
                                                 lhsT=w0x_sb[:, m * 128:(m + 1) * 128],
                                                 rhs=xin, start=False, stop=False,
                                                 skip_group_check=True)
                            for m in range(8):
                                nc.tensor.matmul(
                                    g0[:, m, :],
                                    lhsT=wh0_sb[:, :, m * 128:(m + 1) * 128],
                                    rhs=h0T, start=False, stop=True,
                                    perf_mode=mybir.MatmulPerfMode.DoubleRow,
                                    skip_group_check=True)
                        elif cfg["dstart"]:
                            # even-slot x-matmul streams [xin | zeros] N=512:
                            # bank-wide start writing W@x + zeros in one go;
                            # odd-slot x-matmul then accumulates N=256. Bias
                            # rides the xs ones-row (w0x row E).
                            for bk in range(4):
                                m = 2 * bk
                                nc.tensor.matmul(g0[:, m:m + 2, :],
                                                 lhsT=w0x_sb[:, m * 128:(m + 1) * 128],
                                                 rhs=xin2[:, sl, :], start=True,
                                                 stop=False, skip_group_check=True)
                            for bk in range(4):
                                m = 2 * bk + 1
                                nc.tensor.matmul(g0[:, m, :],
                                                 lhsT=w0x_sb[:, m * 128:(m + 1) * 128],
                                                 rhs=xin2[:, sl, 0:BL], start=False,
                                                 stop=False, skip_group_check=True)
                            # slot-major so sigmoid's slots 0:6 finish early
                            for m in range(8):
                                for kc in range(2):
                                    nc.tensor.matmul(g0[:, m, :],
                                                     lhsT=wh0_sb[:, kc, m * 128:(m + 1) * 128],
                                                     rhs=h0T[:, kc, :], start=False,
                                                     stop=(kc == 1), skip_group_check=True)
                        else:
                            for bk in range(4):
                                nc.tensor.matmul(g0[:, 2 * bk:2 * bk + 2, :],
                                                 lhsT=b0p_sb[:, bk, :],
                                                 rhs=sel2_sb, start=True, stop=False,
                                                 skip_group_check=True)
                            for m in range(8):
                                nc.tensor.matmul(g0[:, m, :],
                                                 lhsT=w0x_sb[0:E, m * 128:(m + 1) * 128],
                                                 rhs=xin, start=False,
                                                 stop=False, skip_group_check=True)
                            for kc in range(2):
                                for m in range(8):
                                    nc.tensor.matmul(g0[:, m, :],
                                                     lhsT=wh0_sb[:, kc, m * 128:(m + 1) * 128],
                                                     rhs=h0T[:, kc, :], start=False,
                                                     stop=(kc == 1), skip_group_check=True)
                    if do1:
                        g1 = pg1.tile([128, 8, BL], FP32, tag="g1")
                        for bk in range(4):
                            nc.tensor.matmul(g1[:, 2 * bk:2 * bk + 2, :],
                                             lhsT=b1p_sb[:, bk, :],
                                             rhs=sel2_sb, start=True, stop=False,
                                             skip_group_check=True)
                        # wi1 @ h0(it-1) first (h0T still holds it-1 here)
                        if fp8:
                            for m in range(8):
                                nc.tensor.matmul(
                                    g1[:, m, :],
                                    lhsT=wi1_sb[:, :, m * 128:(m + 1) * 128],
                                    rhs=h0T, start=False, stop=False,
                                    perf_mode=mybir.MatmulPerfMode.DoubleRow,
                                    skip_group_check=True)
                        else:
                            for m in range(8):
                                for kc in range(2):
                                    nc.tensor.matmul(g1[:, m, :],
                                                     lhsT=wi1_sb[:, kc, m * 128:(m + 1) * 128],
                                                     rhs=h0T[:, kc, :], start=False,
                                                     stop=False, skip_group_check=True)
                        # wh1 @ h1(it-2): h1T write for it-1 lands late, so last
                        for m in range(8):
                            for kc in range(2):
                                nc.tensor.matmul(g1[:, m, :],
                                                 lhsT=wh1_sb[:, kc, m * 128:(m + 1) * 128],
                                                 rhs=h1T[:, kc, :], start=False,
                                                 stop=(kc == 1), skip_group_check=True)

                    # --- scalar/vector queues (overlap next iter's matmuls)
                    if do0:
                        if cfg["debug"] and it == 0:
                            g0cp = pt.tile([128, 8, BL], FP32, tag="g0cp")
                            nc.vector.tensor_copy(g0cp, g0)
                            _dma(nc, dbg_g0_d[:, :, :], g0cp)
                        lstm_layer(g0, pt, h0T, c0, HS if fp8 else 1.0)
                        if cfg["debug"]:
                            h0cp = pt.tile([128, 2, BL], FP32, tag="h0cp")
                            nc.vector.tensor_copy(h0cp, h0T)
                            _dma(nc, dbg_h0_d[it, :, :, :].rearrange("h p b -> p h b"),
                                 h0cp)
                    if do1:
                        lstm_layer(g1, pt, h1T, c1, 1.0)
                        if cfg["debug"]:
                            _dma(nc, hs_d[it - 1, :, :, :].rearrange("h p b -> p h b"),
                                 h1T)
                        else:
                            _dma(nc, hs_ds[(it - 1) // TCH][(it - 1) % TCH, :, :, :]
                                 .rearrange("h p b -> p h b"), h1T)

            # =========================== P3: head ===========================
            if "3" in cfg["phases"]:
              with tc.tile_pool(name="p3in", bufs=1) as pin, \
                 tc.tile_pool(name="p3t", bufs=4) as ptmp, \
                 tc.tile_pool(name="p3ps", bufs=6, space="PSUM") as pps:
                hwt_sb = pin.tile([128, 2, 20], BF16)
                _dma(nc, hwt_sb, hwt_d[:, :].rearrange("(c p) k -> p c k", p=128))
                hb16_sb = pin.tile([1, 2 * TCH * 20], BF16)
                _dma(nc, hb16_sb, hb16_d[:, :])
                ones1b = pin.tile([1, 128], BF16)
                nc.vector.memset(ones1b, 1.0)
                iotak_sb = pin.tile([128, 20], BF16)
                _dma(nc, iotak_sb, iotak_d[:, :])
                xb_sb = pin.tile([128, 2, D], BF16)
                _dma(nc, xb_sb, xb_d[:, :, :])
                # se values for all (hf, t): ln deferred to one batched pass so
                # the Exp/Ln activation tables are not reloaded per chunk
                se_all = pin.tile([128, 2, D], FP32)
                gth_all = pin.tile([128, 2, NCH], FP32)

                # one-hot(x) masks for ALL (hf, t, k) in a single broadcast
                # compare, off the per-chunk critical chain
                mask_all = pin.tile([128, 2, D, 20], BF16)
                xbc = bass.AP(tensor=xb_sb.tensor, offset=xb_sb.offset,
                              ap=[list(d) for d in xb_sb[:, :, :].ap] + [[0, 20]])
                iob = bass.AP(
                    tensor=iotak_sb.tensor, offset=iotak_sb.offset,
                    ap=[list(iotak_sb[:, :].ap[0]), [0, 2], [0, D], [1, 20]])
                nc.vector.tensor_tensor(mask_all, xbc, iob, ALU.is_equal)

                for ci in range(NCH):
                    t0 = ci * TCH
                    hsb = ptmp.tile([128, 2, TCH, BL], BF16, tag="hsb")
                    for kc in range(2):
                        # kc=0 on gpsimd so it prefetches during P2 (the sync
                        # queue is paced by P2's per-iteration stores)
                        eng = nc.gpsimd if kc == 0 else nc.sync
                        if cfg["debug"]:
                            src_ap = _ap(hs_d, (t0 * 2 + kc) * 128 * BL,
                                         [[BL, 128], [2 * 128 * BL, TCH], [1, BL]])
                        else:
                            src_ap = _ap(hs_ds[ci], kc * 128 * BL,
                                         [[BL, 128], [2 * 128 * BL, TCH], [1, BL]])
                        eng.dma_start(out=hsb[:, kc, :, :], in_=src_ap)
                    # one PSUM bank holds logits for both batch halves
                    ps_lg = pps.tile([128, 2, TCH, 20], FP32, tag="ps_lg")
                    nc.tensor.matmul(ps_lg[:, :, :, :], lhsT=ones1b, rhs=hb16_sb,
                                     start=True, stop=False, skip_group_check=True)
                    for hf in range(2):
                        for tt in range(TCH):
                            for kc in range(2):
                                nc.tensor.matmul(
                                    ps_lg[:, hf, tt, :],
                                    lhsT=hsb[:, kc, tt, hf * 128:(hf + 1) * 128],
                                    rhs=hwt_sb[:, kc, :],
                                    start=False, stop=(kc == 1), skip_group_check=True)
                    elg = ptmp.tile([128, 2, TCH, 20], FP32, tag="elg")
                    nc.scalar.activation(elg, ps_lg, AF.Exp)
                    nc.vector.tensor_reduce(se_all[:, :, t0:t0 + TCH], elg,
                                            axis=mybir.AxisListType.X, op=ALU.add)
                    # fused multiply+reduce of precomputed one-hot masks
                    # against the logits per half
                    scr = ptmp.tile([128, TCH, 20], FP32, tag="scr")
                    for hf in range(2):
                        nc.vector.scalar_tensor_tensor(
                            out=scr, in0=ps_lg[:, hf], scalar=1.0,
                            in1=mask_all[:, hf, t0:t0 + TCH, :],
                            op0=ALU.bypass, op1=ALU.mult,
                            accum_out=gth_all[:, hf, ci:ci + 1])

                lse2 = pin.tile([128, 2], FP32)
                for hf in range(2):
                    lnall = pin.tile([128, D], FP32, name=f"lnall_{hf}")
                    nc.scalar.activation(lnall, se_all[:, hf, :], AF.Ln,
                                         accum_out=lse2[:, hf:hf + 1])
                gth2 = pin.tile([128, 2], FP32)
                nc.vector.tensor_reduce(gth2, gth_all[:, :, :],
                                        axis=mybir.AxisListType.X, op=ALU.add)
                res = pin.tile([128, 2], FP32)
                nc.vector.tensor_sub(res, gth2, lse2)
                _dma(nc, _ap(logq_d, 0, [[1, 128], [128, 2]]), res)

    nc.compile()
    return nc


def _prep(inputs, cfg):
    f32 = np.float32
    emb = np.asarray(inputs["emb"], f32)
    perm = np.r_[0:2 * H, 3 * H:4 * H, 2 * H:3 * H]   # torch i,f,g,o -> i,f,o,g

    def pair_bias(b):
        # b[G4] -> [2, 4, 128]: b0p[j, bk, p] = b[(2bk+j)*128+p]
        return b.reshape(4, 2, 128).transpose(1, 0, 2).copy()

    wih0 = np.asarray(inputs["wih0"], f32)
    whh0 = np.asarray(inputs["whh0"], f32)
    wih1 = np.asarray(inputs["wih1"], f32)
    whh1 = np.asarray(inputs["whh1"], f32)
    b0 = (np.asarray(inputs["bih0"], f32) + np.asarray(inputs["bhh0"], f32))[perm]
    b1 = (np.asarray(inputs["bih1"], f32) + np.asarray(inputs["bhh1"], f32))[perm]

    emb2 = np.zeros((40, 128), f32)
    emb2[:20, :64] = emb
    emb2[20:, 64:] = emb
    sel2 = np.zeros((2, 512), f32)
    sel2[0, :256] = 1.0
    sel2[1, 256:] = 1.0
    gb2 = np.asarray(inputs["gb2"], f32)
    bb2 = np.asarray(inputs["bb2"], f32)

    shared = {
        "gw1t": np.ascontiguousarray(np.asarray(inputs["gw1"], f32).T),
        "bw1t": np.ascontiguousarray(np.asarray(inputs["bw1"], f32).T),
        "gb1": np.asarray(inputs["gb1"], f32),
        "bb1": np.asarray(inputs["bb1"], f32),
        "glnw": np.asarray(inputs["gln_w"], f32),
        "glnb": np.asarray(inputs["gln_b"], f32),
        "blnw": np.asarray(inputs["bln_w"], f32),
        "blnb": np.asarray(inputs["bln_b"], f32),
        "gw2t": np.ascontiguousarray(np.asarray(inputs["gw2"], f32).T),
        "bw2t": np.ascontiguousarray(np.asarray(inputs["bw2"], f32).T),
        "gb2d": np.concatenate([gb2, gb2])[:, None].copy(),
        "bb2d": np.concatenate([bb2, bb2])[:, None].copy(),
        "u0wt": np.ascontiguousarray(np.asarray(inputs["u0_w"], f32).T),
        "u0b": np.asarray(inputs["u0_b"], f32)[:, None].copy(),
        "ident": np.eye(128, dtype=f32),
        "emb2": emb2.astype(BF),
        "iota2": (np.arange(40, dtype=f32) % 20)[:, None].copy(),
        "w0x": np.ascontiguousarray(np.vstack([wih0.T[:, perm], b0[None, :]])).astype(BF),
        "wh0": np.ascontiguousarray(whh0.T[:, perm]).astype(BF),
        "wi1": np.ascontiguousarray(wih1.T[:, perm]).astype(BF),
        "wh1": np.ascontiguousarray(whh1.T[:, perm]).astype(BF),
        "b0p": pair_bias(b0).astype(BF),
        "b1p": pair_bias(b1).astype(BF),
        # fp8 path: gate pre-activations carry a GS=256 scale (wh0/wi1 are
        # fp8 x16 and h0 is stored as fp8 x16; bf16 contributions are x256)
        "w0xs": (np.vstack([wih0.T[:, perm], b0[None, :]]) * GS).astype(BF),
        "wh08": np.ascontiguousarray(
            (whh0.T[:, perm] * HS).reshape(2, 128, G4).transpose(1, 0, 2)).astype(F8),
        "wi18": np.ascontiguousarray(
            (wih1.T[:, perm] * HS).reshape(2, 128, G4).transpose(1, 0, 2)).astype(F8),
        "wh1s": np.ascontiguousarray(
            (whh1.T[:, perm] * GS).reshape(2, 128, G4).transpose(1, 0, 2)).astype(BF),
        "b1ps": (pair_bias(b1) * GS).astype(BF),
        "sel2": sel2.astype(BF),
        "hwt": np.ascontiguousarray(np.asarray(inputs["head_w"], f32).T).astype(BF),
        "hb8": np.tile(np.asarray(inputs["head_b"], f32), TCH)[None, :].astype(BF),
        "hb16": np.tile(np.asarray(inputs["head_b"], f32), 2 * TCH)[None, :].astype(BF),
        "iotak": np.tile(np.arange(20, dtype=f32), (128, 1)).astype(BF),
    }

    U = np.asarray(inputs["U"], f32)
    X = np.asarray(inputs["X"])
    in_maps = []
    for c in range(N_CORES):
        Us = U[c * BL:(c + 1) * BL]
        Xs = X[c * BL:(c + 1) * BL]
        m = dict(shared)
        m["ut"] = np.ascontiguousarray(Us.T)
        m["xt"] = np.ascontiguousarray(Xs.T.astype(f32)).astype(BF)
        m["xb"] = np.ascontiguousarray(
            Xs.astype(f32).reshape(2, 128, D).transpose(1, 0, 2)).astype(BF)
        in_maps.append(m)
    return in_maps


def get_nc(cfg=None):
    cfg = dict(CFG, **(cfg or {}))
    key = tuple(sorted(cfg.items()))
    if key not in _CACHE:
        _CACHE[key] = build_nc(cfg)
    return _CACHE[key]


def run(inputs, cfg=None, **run_kwargs):
    cfg = dict(CFG, **(cfg or {}))
    nc = get_nc(cfg)
    in_maps = _prep(inputs, cfg)
    res = run_bass_kernel_spmd(nc, in_maps, core_ids=list(range(N_CORES)), **run_kwargs)
    return res


def kernel(**inputs):
    res = run(inputs)
    out = np.concatenate([res.results[c]["logq"] for c in range(N_CORES)])
    return out.astype(np.float32)

